# revision 27
# baseline (speedup 1.0000x reference)
"""Trainium2 Bass kernel for nn_PredictionNet — data-parallel over batch.

Each of the 8 cores handles a 32-sample batch slice with ALL expert weights
resident in SBUF. No cross-core communication.

Two structural optimizations over the fp16 per-expert-blend version:

1. fp8 (e4m3) for every large weight tensor. The network re-injects the
   unit-scale z features at layers 2/3, so the hidden-path weights (w1, w2,
   w3 hidden rows) carry only ~0.4% of the output variance headroom and
   tolerate fp8 easily (measured 2.1e-3 absmax-rel vs f64). The z/bias rows
   (w2z, w3z) stay fp16. Weights are pre-scaled by 256 (acts by 16/64) on
   the host to live in the e4m3 normal range; descales fold into the ELU /
   final-copy scalar ops. Layer 1 runs DoubleRow fp8 matmuls (K=256 per
   instruction, 2x PE throughput).

2. Blend-free PSUM accumulation: the per-sample blend coefficient c[b,e]
   is folded into the *stationary* matmul operands (host-scaled hc6/zc3/c6;
   diag-scaled transpose for h1/h2), so the expert sum accumulates directly
   in PSUM. This removes all 24 per-expert DVE blend ops of the old design.
   The transpose+scale of h between layers is a single regular matmul per
   128-chunk against a [32, 6*32] block-of-diagonals operand.
"""

import sys

sys.path.insert(0, "/opt/trn_rl_repo")

import numpy as np
import ml_dtypes

import concourse.bass as bass
import concourse.mybir as mybir
import concourse.tile as tile
from concourse.bass_utils import run_bass_kernel_spmd

B, E = 256, 6
IN, HID, OUT, ZD = 1664, 512, 618, 32
N_CORES = 8
CORE_IDS = list(range(N_CORES))
BC = B // N_CORES         # 32 batch rows per core
K1 = IN // 128            # 13 real k-chunks, layer 1
K1P = 14                  # padded to 14 = 7 DoubleRow pairs (slot 13 zeroed)
KH = HID // 128           # 4 k-chunks for the hidden part of layers 2/3
OUTP = 640                # layer-3 output padded 618 -> 640
NH3 = 2                   # layer-3 output split into halves of 320 (psum bank)
OH3 = OUTP // NH3
ZR = 1 + ZD               # 33 rows: ones row (bias) + z
ZK = 3 * ZR               # 99: three experts' z-blocks stacked per group
FP32 = mybir.dt.float32
FP16 = mybir.dt.float16
FP8 = mybir.dt.float8e4
E4 = ml_dtypes.float8_e4m3
DR = mybir.MatmulPerfMode.DoubleRow

SH0 = 16.0                # h0 pre-scale into fp8
SW = 256.0                # weight pre-scale into fp8
SH = 64.0                 # hidden-act scale (folded into the diag operand)
SZ1 = SH0 * SW            # layer-1 psum scale
SZ2 = SH * SW             # layer-2/3 psum scale


def _split_waits(nc, max_waits=1):
    """neuronxcc walrus accepts only ONE sync-wait per instruction: hoist
    extras onto same-engine NoOps placed before the offending instruction."""
    n = 0
    for fn in nc.m.functions:
        for blk in fn.blocks:
            insts = blk.instructions
            if not any(
                i.sync_info is not None and len(i.sync_info.on_wait) > max_waits
                for i in insts
            ):
                continue
            out = []
            for inst in insts:
                si = inst.sync_info
                if si is not None and len(si.on_wait) > max_waits:
                    for w in si.on_wait[:-max_waits]:
                        n += 1
                        nop = mybir.InstNoOp(name=f"I-wfix{n}", ins=[], outs=[])
                        nop.engine = inst.engine
                        nop.sync_info = mybir.SyncInfo(on_wait=[w], on_update=[])
                        try:
                            nc.register_instruction(nop, overwrite=True)
                        except Exception:
                            pass
                        out.append(nop)
                    inst.sync_info = mybir.SyncInfo(
                        on_wait=list(si.on_wait[-max_waits:]),
                        on_update=list(si.on_update),
                    )
                out.append(inst)
            blk.instructions = out
    return n


def build_nc():
    nc = bass.Bass()

    hc6_d = nc.dram_tensor("hc6", [128, E, K1P, BC], FP8, kind="ExternalInput")
    c6_d = nc.dram_tensor("c6", [E, BC], FP16, kind="ExternalInput")
    zc3_d = nc.dram_tensor("zc3", [ZK, 2, BC], FP16, kind="ExternalInput")
    dg6_d = nc.dram_tensor("dg6", [BC, E * BC], FP16, kind="ExternalInput")
    b1s_d = nc.dram_tensor("b1s", [E, HID], FP16, kind="ExternalInput")
    w1q_d = nc.dram_tensor("w1q", [E, 128, K1, HID], FP8, kind="ExternalInput")
    w2zs_d = nc.dram_tensor("w2zs", [ZK, 2, HID], FP16, kind="ExternalInput")
    w2q_d = nc.dram_tensor("w2q", [E, 128, KH, HID], FP8, kind="ExternalInput")
    w3zs_d = nc.dram_tensor("w3zs", [ZK, 2, OUTP], FP16, kind="ExternalInput")
    w3q_d = nc.dram_tensor("w3q", [E, 128, KH, OUTP], FP8, kind="ExternalInput")
    out_d = nc.dram_tensor("outc", [BC, OUTP], FP32, kind="ExternalOutput")

    with tile.TileContext(nc) as tc:
        with (
            tc.tile_pool(name="const", bufs=1) as cp,
            tc.tile_pool(name="work", bufs=1) as wp,
            tc.tile_pool(name="psum", bufs=3, space="PSUM") as pp,
            tc.tile_pool(name="psumt", bufs=4, space="PSUM") as pt,
        ):
            # ---- DMA schedule. Critical-path-first on the two fast HWDGE
            # queues: layer-1 operands (hc6 on sync; c6/b1s on scalar) ahead
            # of the weight slabs; w1 expert halves interleaved across both
            # queues in PE-consumption order. Slack tensors ride SWDGE.
            hc6 = cp.tile([128, E, K1P, BC], FP8)
            nc.sync.dma_start(out=hc6[:], in_=hc6_d[:])
            c6 = cp.tile([E, BC], FP16)
            nc.scalar.dma_start(out=c6[:], in_=c6_d[:])
            b1s = cp.tile([E, HID], FP16)
            nc.scalar.dma_start(out=b1s[:], in_=b1s_d[:])
            # w1 expert slabs split in k-halves in PE-consumption order:
            # a-halves (k0..7, 0.52MB) on sync, b-halves (k8..12, 0.33MB)
            # on scalar. The scalar queue measurably starts moving ~4us
            # after sync, so the ~3.5/2.0 MB byte split makes both queues
            # finish w1 near-simultaneously (~23us).
            w1 = cp.tile([128, E, K1P, HID], FP8)
            # zero the padded 14th k-slot once; DoubleRow pair 6 reads it
            nc.vector.memset(w1[:, :, K1P - 1, :], 0.0)
            KA = 8
            for e in range(E):
                nc.sync.dma_start(
                    out=w1[:, e, :KA, :], in_=w1q_d[e, :, :KA, :]
                )
                nc.scalar.dma_start(
                    out=w1[:, e, KA:K1, :], in_=w1q_d[e, :, KA:, :]
                )
            # ---- layer-2/3 tensors spread over all three queues (SWDGE
            # wakes early and is otherwise idle after the small tensors) ----
            zc3 = cp.tile([ZK, 2, BC], FP16)
            nc.gpsimd.dma_start(out=zc3[:], in_=zc3_d[:])
            dg6 = cp.tile([BC, E * BC], FP16)
            nc.gpsimd.dma_start(out=dg6[:], in_=dg6_d[:])
            w2zs = cp.tile([ZK, 2, HID], FP16)
            nc.gpsimd.dma_start(out=w2zs[:], in_=w2zs_d[:])
            w2 = cp.tile([128, E, KH, HID], FP8)
            w3 = cp.tile([128, E, KH, OUTP], FP8)
            nc.gpsimd.dma_start(out=w2[:, 4, :, :], in_=w2q_d[4])
            nc.gpsimd.dma_start(out=w2[:, 5, :, :], in_=w2q_d[5])
            w3zs = cp.tile([ZK, 2, OUTP], FP16)
            nc.gpsimd.dma_start(out=w3zs[:], in_=w3zs_d[:])
            nc.gpsimd.dma_start(out=w3[:, 4, :, :], in_=w3q_d[4])
            nc.gpsimd.dma_start(out=w3[:, 5, :, :], in_=w3q_d[5])
            for e in (0, 2):
                nc.sync.dma_start(out=w2[:, e, :, :], in_=w2q_d[e])
            for e in (1, 3):
                nc.scalar.dma_start(out=w2[:, e, :, :], in_=w2q_d[e])
            for e in (0, 2):
                nc.sync.dma_start(out=w3[:, e, :, :], in_=w3q_d[e])
            for e in (1, 3):
                nc.scalar.dma_start(out=w3[:, e, :, :], in_=w3q_d[e])

            HH = HID // 2     # 256-column half of a hidden layer

            def elu_half(ps, s, h, c0, tag):
                """h[:, c0:c0+HH] = ELU(ps / s) in fp16 via
                relu(x) + min(exp(x)-1, 0). exp/relu read PSUM directly on
                ACT (descale fused via scale=); x is O(1) so exp is safe."""
                texp = wp.tile([BC, HH], FP16, tag=f"{tag}_exp")
                nc.scalar.activation(
                    texp[:], ps[:], mybir.ActivationFunctionType.Exp,
                    scale=1.0 / s,
                )
                trel = wp.tile([BC, HH], FP16, tag=f"{tag}_rel")
                nc.scalar.activation(
                    trel[:], ps[:], mybir.ActivationFunctionType.Relu,
                    scale=1.0 / s,
                )
                tmin = wp.tile([BC, HH], FP16, tag=f"{tag}_min")
                nc.vector.tensor_scalar(
                    tmin[:], texp[:], -1.0, 0.0,
                    mybir.AluOpType.add, mybir.AluOpType.min,
                )
                nc.vector.tensor_tensor(
                    h[:, c0 : c0 + HH], tmin[:], trel[:], mybir.AluOpType.add
                )

            def tscale_half(h, ht, half, tag):
                """ht[p, e, j, b] = 64 * c[b,e] * h[b, 128j+p] for the two
                128-chunks j of column-half `half`. One regular matmul per
                chunk against the block-of-diags dg6 [32, 6*32] does
                transpose + per-expert coef scaling; PSUM->SBUF fp8 casts
                ride the ACT engine (DVE is on ELU duty)."""
                for j in range(2 * half, 2 * half + 2):
                    ps = pt.tile([128, E, BC], FP32, name=f"{tag}_tp{j}", tag="tpose")
                    nc.tensor.matmul(
                        ps[:], h[:, j * 128 : (j + 1) * 128], dg6[:],
                        start=True, stop=True,
                    )
                    nc.scalar.activation(
                        ht[:, :, j, :], ps[:], mybir.ActivationFunctionType.Copy
                    )

            # All three layers are split into two output-column half-chains
            # (A = first half, B = second half) so the ELU + transpose of
            # half A overlaps PE work of half B — the PE stream never has a
            # multi-us bubble and the gated clock stays at 2.4 GHz.

            # ================= Layer 1 (DoubleRow fp8, 7 pairs) =============
            ps1h = [
                pp.tile([BC, HH], FP32, name=f"l1ps{h}", tag="ps")
                for h in range(2)
            ]
            # a-phase (k-pairs 0..3, arriving on sync) over all experts
            # first, then b-phase (pairs 4..6, on the late-waking scalar
            # queue) — PE consumption order matches DMA arrival order.
            QSPANS = [(0, KA // 2), (KA // 2, K1P // 2)]  # pair ranges a / b
            for (j0, j1) in QSPANS:
                for e in range(E):
                    for h in range(2):
                        sl = slice(h * HH, (h + 1) * HH)
                        for j in range(j0, j1):
                            nc.tensor.matmul(
                                ps1h[h][:],
                                hc6[:, e, 2 * j : 2 * j + 2, :],
                                w1[:, e, 2 * j : 2 * j + 2, sl],
                                start=(e == 0 and j == 0), stop=False,
                                perf_mode=DR,
                            )
            h1 = wp.tile([BC, HID], FP16, tag="l1_h")
            h1t = wp.tile([128, E, KH, BC], FP8, tag="l1_t")
            for h in range(2):
                sl = slice(h * HH, (h + 1) * HH)
                # blended bias last: += sum_e c[b,e] * (SZ1*b1[e, half])
                nc.tensor.matmul(
                    ps1h[h][:], c6[:], b1s[:, sl], start=False, stop=True
                )
                elu_half(ps1h[h], SZ1, h1, h * HH, f"l1{h}")
                tscale_half(h1, h1t, h, f"l1{h}")

            # ================= Layer 2 (DoubleRow fp8) =================
            ps2h = [
                pp.tile([BC, HH], FP32, name=f"l2ps{h}", tag="ps")
                for h in range(2)
            ]
            for h in range(2):
                sl = slice(h * HH, (h + 1) * HH)
                for g in range(2):
                    nc.tensor.matmul(
                        ps2h[h][:], zc3[:, g, :], w2zs[:, g, sl],
                        start=(g == 0), stop=False,
                    )
            h2 = wp.tile([BC, HID], FP16, tag="l2_h")
            h2t = wp.tile([128, E, KH, BC], FP8, tag="l2_t")
            # pair j contracts h1 columns [256j, 256j+256) = tscale half j
            for j in range(KH // 2):
                for e in range(E):
                    for h in range(2):
                        sl = slice(h * HH, (h + 1) * HH)
                        nc.tensor.matmul(
                            ps2h[h][:],
                            h1t[:, e, 2 * j : 2 * j + 2, :],
                            w2[:, e, 2 * j : 2 * j + 2, sl],
                            start=False,
                            stop=(e == E - 1 and j == KH // 2 - 1),
                            perf_mode=DR,
                        )
            for h in range(2):
                elu_half(ps2h[h], SZ2, h2, h * HH, f"l2{h}")
                tscale_half(h2, h2t, h, f"l2{h}")

            # ================= Layer 3 (DoubleRow fp8) =================
            res3 = wp.tile([BC, OUTP], FP32, tag="res3")
            ps3h = [
                pp.tile([BC, OH3], FP32, name=f"l3ps{h}", tag="ps")
                for h in range(2)
            ]
            for h in range(2):
                sl = slice(h * OH3, (h + 1) * OH3)
                for g in range(2):
                    nc.tensor.matmul(
                        ps3h[h][:], zc3[:, g, :], w3zs[:, g, sl],
                        start=(g == 0), stop=False,
                    )
            for j in range(KH // 2):
                for e in range(E):
                    for h in range(2):
                        sl = slice(h * OH3, (h + 1) * OH3)
                        nc.tensor.matmul(
                            ps3h[h][:],
                            h2t[:, e, 2 * j : 2 * j + 2, :],
                            w3[:, e, 2 * j : 2 * j + 2, sl],
                            start=False,
                            stop=(e == E - 1 and j == KH // 2 - 1),
                            perf_mode=DR,
                        )
            for h in range(2):
                sl = slice(h * OH3, (h + 1) * OH3)
                nc.vector.tensor_scalar(
                    res3[:, sl], ps3h[h][:], 1.0 / SZ2, None,
                    mybir.AluOpType.mult,
                )
                # stream each half out as soon as its copy lands
                eng = nc.scalar if h == 0 else nc.sync
                eng.dma_start(out=out_d[:, sl], in_=res3[:, sl])

    _split_waits(nc)
    _trim_tail(nc)
    return nc


def _trim_tail(nc):
    """Drop the second all-engine barrier round + sem-clear at the kernel
    tail: the first drain+barrier already guarantees completion, and the
    preamble re-initializes semaphores on any re-execution (verified by
    double-execution test)."""
    blk = nc.m.functions[0].blocks[-1]
    insts = blk.instructions
    cut = None
    for idx in range(len(insts) - 1, -1, -1):
        if type(insts[idx]).__name__ == "InstISA":
            cut = idx
            break
    if cut is not None:
        blk.instructions = insts[:cut]


_NC_CACHE = None


def _get_nc():
    global _NC_CACHE
    if _NC_CACHE is None:
        _NC_CACHE = build_nc()
    return _NC_CACHE


def _zgroup(wz, width):
    """[E, 33, width] (bias row + z rows, pre-scaled) -> [99, 2, width]
    where row 33e+r of group g holds expert 3g+e's row r."""
    t = wz.reshape(2, 3, ZR, width).transpose(1, 2, 0, 3)
    return np.ascontiguousarray(t.reshape(ZK, 2, width))


def make_in_maps(p_prev, blending_coef, z, w_l1, b_l1, w_l2, b_l2, w_l3, b_l3):
    f, h = np.float32, np.float16
    h0 = np.concatenate([z, p_prev], axis=1).astype(f)            # [B, IN]
    coef = blending_coef.astype(f)

    w1q = np.ascontiguousarray(                                    # [E,128,K1,HID]
        (SW * w_l1.astype(f)).astype(E4)
        .reshape(E, K1, 128, HID).transpose(0, 2, 1, 3)
    )
    b1s = (SZ1 * b_l1.astype(f)).astype(h)                         # [E, HID]
    w2z = np.concatenate(
        [b_l2.astype(f)[:, None, :], w_l2[:, :ZD, :].astype(f)], axis=1
    )                                                              # [E, 33, HID]
    w2zs = _zgroup((SZ2 * w2z).astype(h), HID)                     # [99, 2, HID]
    w2q = np.ascontiguousarray(                                    # [E,128,KH,HID]
        (SW * w_l2[:, ZD:, :].astype(f)).astype(E4)
        .reshape(E, KH, 128, HID).transpose(0, 2, 1, 3)
    )
    w3p = np.zeros((E, HID + ZD, OUTP), f)
    w3p[:, :, :OUT] = w_l3
    b3p = np.zeros((E, OUTP), f)
    b3p[:, :OUT] = b_l3
    w3z = np.concatenate([b3p[:, None, :], w3p[:, :ZD, :]], axis=1)
    w3zs = _zgroup((SZ2 * w3z).astype(h), OUTP)                    # [99, 2, OUTP]
    w3q = np.ascontiguousarray(                                    # [E,128,KH,OUTP]
        (SW * w3p[:, ZD:, :]).astype(E4)
        .reshape(E, KH, 128, OUTP).transpose(0, 2, 1, 3)
    )
    eye = np.eye(BC, dtype=f)

    in_maps = []
    for c in range(N_CORES):
        bs = slice(c * BC, (c + 1) * BC)
        cc = coef[bs]                                              # [BC, E]
        hv = h0[bs].T.reshape(K1, 128, BC)                         # [k, p, b]
        hc6 = np.zeros((128, E, K1P, BC), E4)                      # k13 stays 0
        hc6[:, :, :K1, :] = (
            SH0 * np.einsum("be,kpb->pekb", cc, hv)
        ).astype(E4)
        c6 = np.ascontiguousarray(cc.T).astype(h)                  # [E, BC]
        zext = np.concatenate(
            [np.ones((BC, 1), f), z[bs].astype(f)], axis=1
        ).T                                                        # [33, BC]
        zc3t = np.einsum("rb,bE->Erb", zext, cc)                   # [6, 33, BC]
        zc3 = np.ascontiguousarray(
            zc3t.reshape(2, 3, ZR, BC).transpose(1, 2, 0, 3).reshape(ZK, 2, BC)
        ).astype(h)
        dg6 = np.ascontiguousarray(                                # [BC, E*BC]
            np.einsum("be,bc->bec", SH * cc, eye).reshape(BC, E * BC)
        ).astype(h)
        in_maps.append(
            {
                "hc6": hc6, "c6": c6, "zc3": zc3, "dg6": dg6, "b1s": b1s,
                "w1q": w1q, "w2zs": w2zs, "w2q": w2q,
                "w3zs": w3zs, "w3q": w3q,
            }
        )
    return in_maps


def assemble_output(results):
    full = np.concatenate(
        [results[c]["outc"] for c in range(N_CORES)], axis=0
    )                                                              # [256, 640]
    return np.ascontiguousarray(full[:, :OUT]).astype(np.float32)


def kernel(p_prev, blending_coef, z, w_l1, b_l1, w_l2, b_l2, w_l3, b_l3):
    args = [
        np.asarray(a)
        for a in (p_prev, blending_coef, z, w_l1, b_l1, w_l2, b_l2, w_l3, b_l3)
    ]
    nc = _get_nc()
    in_maps = make_in_maps(*args)
    res = run_bass_kernel_spmd(nc, in_maps, CORE_IDS)
    return assemble_output(res.results)


# revision 28
# speedup vs baseline: 1.0912x; 1.0912x over previous
"""Trainium2 Bass kernel for nn_PredictionNet — data-parallel over batch.

Each of the 8 cores handles a 32-sample batch slice with ALL expert weights
resident in SBUF. No cross-core communication.

Two structural optimizations over the fp16 per-expert-blend version:

1. fp8 (e4m3) for every large weight tensor. The network re-injects the
   unit-scale z features at layers 2/3, so the hidden-path weights (w1, w2,
   w3 hidden rows) carry only ~0.4% of the output variance headroom and
   tolerate fp8 easily (measured 2.1e-3 absmax-rel vs f64). The z/bias rows
   (w2z, w3z) stay fp16. Weights are pre-scaled by 256 (acts by 16/64) on
   the host to live in the e4m3 normal range; descales fold into the ELU /
   final-copy scalar ops. Layer 1 runs DoubleRow fp8 matmuls (K=256 per
   instruction, 2x PE throughput).

2. Blend-free PSUM accumulation: the per-sample blend coefficient c[b,e]
   is folded into the *stationary* matmul operands (host-scaled hc6/zc3/c6;
   diag-scaled transpose for h1/h2), so the expert sum accumulates directly
   in PSUM. This removes all 24 per-expert DVE blend ops of the old design.
   The transpose+scale of h between layers is a single regular matmul per
   128-chunk against a [32, 6*32] block-of-diagonals operand.
"""

import sys

sys.path.insert(0, "/opt/trn_rl_repo")

import numpy as np
import ml_dtypes

import concourse.bass as bass
import concourse.mybir as mybir
import concourse.tile as tile
from concourse.bass_utils import run_bass_kernel_spmd

B, E = 256, 6
IN, HID, OUT, ZD = 1664, 512, 618, 32
N_CORES = 8
CORE_IDS = list(range(N_CORES))
BC = B // N_CORES         # 32 batch rows per core
K1 = IN // 128            # 13 real k-chunks, layer 1
K1P = 14                  # padded to 14 = 7 DoubleRow pairs (slot 13 zeroed)
KH = HID // 128           # 4 k-chunks for the hidden part of layers 2/3
OUTP = 640                # layer-3 output padded 618 -> 640
NH3 = 2                   # layer-3 output split into halves of 320 (psum bank)
OH3 = OUTP // NH3
ZR = 1 + ZD               # 33 rows: ones row (bias) + z
ZK = 3 * ZR               # 99: three experts' z-blocks stacked per group
FP32 = mybir.dt.float32
FP16 = mybir.dt.float16
FP8 = mybir.dt.float8e4
E4 = ml_dtypes.float8_e4m3
DR = mybir.MatmulPerfMode.DoubleRow

SH0 = 16.0                # h0 pre-scale into fp8
SW = 256.0                # weight pre-scale into fp8
SH = 64.0                 # hidden-act scale (folded into the diag operand)
SZ1 = SH0 * SW            # layer-1 psum scale
SZ2 = SH * SW             # layer-2/3 psum scale


def _split_waits(nc, max_waits=1):
    """neuronxcc walrus accepts only ONE sync-wait per instruction: hoist
    extras onto same-engine NoOps placed before the offending instruction."""
    n = 0
    for fn in nc.m.functions:
        for blk in fn.blocks:
            insts = blk.instructions
            if not any(
                i.sync_info is not None and len(i.sync_info.on_wait) > max_waits
                for i in insts
            ):
                continue
            out = []
            for inst in insts:
                si = inst.sync_info
                if si is not None and len(si.on_wait) > max_waits:
                    for w in si.on_wait[:-max_waits]:
                        n += 1
                        nop = mybir.InstNoOp(name=f"I-wfix{n}", ins=[], outs=[])
                        nop.engine = inst.engine
                        nop.sync_info = mybir.SyncInfo(on_wait=[w], on_update=[])
                        try:
                            nc.register_instruction(nop, overwrite=True)
                        except Exception:
                            pass
                        out.append(nop)
                    inst.sync_info = mybir.SyncInfo(
                        on_wait=list(si.on_wait[-max_waits:]),
                        on_update=list(si.on_update),
                    )
                out.append(inst)
            blk.instructions = out
    return n


def build_nc():
    nc = bass.Bass()

    hc6_d = nc.dram_tensor("hc6", [128, E, K1P, BC], FP8, kind="ExternalInput")
    c6_d = nc.dram_tensor("c6", [E, BC], FP16, kind="ExternalInput")
    zc3_d = nc.dram_tensor("zc3", [ZK, 2, BC], FP16, kind="ExternalInput")
    dg6_d = nc.dram_tensor("dg6", [BC, E * BC], FP16, kind="ExternalInput")
    b1s_d = nc.dram_tensor("b1s", [E, HID], FP16, kind="ExternalInput")
    w1q_d = nc.dram_tensor("w1q", [E, 128, K1, HID], FP8, kind="ExternalInput")
    w2zs_d = nc.dram_tensor("w2zs", [ZK, 2, HID], FP16, kind="ExternalInput")
    w2q_d = nc.dram_tensor("w2q", [E, 128, KH, HID], FP8, kind="ExternalInput")
    w3zs_d = nc.dram_tensor("w3zs", [ZK, 2, OUTP], FP16, kind="ExternalInput")
    w3q_d = nc.dram_tensor("w3q", [E, 128, KH, OUTP], FP8, kind="ExternalInput")
    out_d = nc.dram_tensor("outc", [BC, OUTP], FP32, kind="ExternalOutput")

    with tile.TileContext(nc) as tc:
        with (
            tc.tile_pool(name="const", bufs=1) as cp,
            tc.tile_pool(name="work", bufs=1) as wp,
            tc.tile_pool(name="psum", bufs=3, space="PSUM") as pp,
            tc.tile_pool(name="psumt", bufs=4, space="PSUM") as pt,
        ):
            # ---- DMA schedule. Critical-path-first on the two fast HWDGE
            # queues: layer-1 operands (hc6 on sync; c6/b1s on scalar) ahead
            # of the weight slabs; w1 expert halves interleaved across both
            # queues in PE-consumption order. Slack tensors ride SWDGE.
            hc6 = cp.tile([128, E, K1P, BC], FP8)
            nc.sync.dma_start(out=hc6[:], in_=hc6_d[:])
            c6 = cp.tile([E, BC], FP16)
            nc.scalar.dma_start(out=c6[:], in_=c6_d[:])
            b1s = cp.tile([E, HID], FP16)
            nc.scalar.dma_start(out=b1s[:], in_=b1s_d[:])
            # w1 expert slabs split in k-halves in PE-consumption order:
            # a-halves (k0..7, 0.52MB) on sync, b-halves (k8..12, 0.33MB)
            # on scalar. The scalar queue measurably starts moving ~4us
            # after sync, so the ~3.5/2.0 MB byte split makes both queues
            # finish w1 near-simultaneously (~23us).
            w1 = cp.tile([128, E, K1P, HID], FP8)
            # zero the padded 14th k-slot once; DoubleRow pair 6 reads it
            nc.vector.memset(w1[:, :, K1P - 1, :], 0.0)
            KA = 8
            for e in range(E):
                nc.sync.dma_start(
                    out=w1[:, e, :KA, :], in_=w1q_d[e, :, :KA, :]
                )
                nc.scalar.dma_start(
                    out=w1[:, e, KA:K1, :], in_=w1q_d[e, :, KA:, :]
                )
            # ---- slack tensors (needed only from layer 2 on) ----
            zc3 = cp.tile([ZK, 2, BC], FP16)
            nc.gpsimd.dma_start(out=zc3[:], in_=zc3_d[:])
            dg6 = cp.tile([BC, E * BC], FP16)
            nc.gpsimd.dma_start(out=dg6[:], in_=dg6_d[:])
            w2zs = cp.tile([ZK, 2, HID], FP16)
            nc.gpsimd.dma_start(out=w2zs[:], in_=w2zs_d[:])
            w3zs = cp.tile([ZK, 2, OUTP], FP16)
            nc.gpsimd.dma_start(out=w3zs[:], in_=w3zs_d[:])
            w2 = cp.tile([128, E, KH, HID], FP8)
            for e in range(E):
                eng = nc.sync if e % 2 == 0 else nc.scalar
                eng.dma_start(out=w2[:, e, :, :], in_=w2q_d[e])
            w3 = cp.tile([128, E, KH, OUTP], FP8)
            for e in range(E):
                eng = nc.sync if e % 2 == 0 else nc.scalar
                eng.dma_start(out=w3[:, e, :, :], in_=w3q_d[e])

            HH = HID // 2     # 256-column half of a hidden layer

            def elu_half(ps, s, h, c0, tag):
                """h[:, c0:c0+HH] = ELU(ps / s) in fp16 via
                relu(x) + min(exp(x)-1, 0). exp/relu read PSUM directly on
                ACT (descale fused via scale=); x is O(1) so exp is safe."""
                texp = wp.tile([BC, HH], FP16, tag=f"{tag}_exp")
                nc.scalar.activation(
                    texp[:], ps[:], mybir.ActivationFunctionType.Exp,
                    scale=1.0 / s,
                )
                trel = wp.tile([BC, HH], FP16, tag=f"{tag}_rel")
                nc.scalar.activation(
                    trel[:], ps[:], mybir.ActivationFunctionType.Relu,
                    scale=1.0 / s,
                )
                tmin = wp.tile([BC, HH], FP16, tag=f"{tag}_min")
                nc.vector.tensor_scalar(
                    tmin[:], texp[:], -1.0, 0.0,
                    mybir.AluOpType.add, mybir.AluOpType.min,
                )
                nc.vector.tensor_tensor(
                    h[:, c0 : c0 + HH], tmin[:], trel[:], mybir.AluOpType.add
                )

            def tscale_half(h, ht, half, tag):
                """ht[p, e, j, b] = 64 * c[b,e] * h[b, 128j+p] for the two
                128-chunks j of column-half `half`. One regular matmul per
                chunk against the block-of-diags dg6 [32, 6*32] does
                transpose + per-expert coef scaling; PSUM->SBUF fp8 casts
                ride the ACT engine (DVE is on ELU duty)."""
                for j in range(2 * half, 2 * half + 2):
                    ps = pt.tile([128, E, BC], FP32, name=f"{tag}_tp{j}", tag="tpose")
                    nc.tensor.matmul(
                        ps[:], h[:, j * 128 : (j + 1) * 128], dg6[:],
                        start=True, stop=True,
                    )
                    nc.scalar.activation(
                        ht[:, :, j, :], ps[:], mybir.ActivationFunctionType.Copy
                    )

            # All three layers are split into two output-column half-chains
            # (A = first half, B = second half) so the ELU + transpose of
            # half A overlaps PE work of half B — the PE stream never has a
            # multi-us bubble and the gated clock stays at 2.4 GHz.

            # ================= Layer 1 (DoubleRow fp8, 7 pairs) =============
            ps1h = [
                pp.tile([BC, HH], FP32, name=f"l1ps{h}", tag="ps")
                for h in range(2)
            ]
            # a-phase (k-pairs 0..3, arriving on sync) over all experts
            # first, then b-phase (pairs 4..6, on the late-waking scalar
            # queue) — PE consumption order matches DMA arrival order.
            QSPANS = [(0, KA // 2), (KA // 2, K1P // 2)]  # pair ranges a / b
            for (j0, j1) in QSPANS:
                for e in range(E):
                    for h in range(2):
                        sl = slice(h * HH, (h + 1) * HH)
                        for j in range(j0, j1):
                            nc.tensor.matmul(
                                ps1h[h][:],
                                hc6[:, e, 2 * j : 2 * j + 2, :],
                                w1[:, e, 2 * j : 2 * j + 2, sl],
                                start=(e == 0 and j == 0), stop=False,
                                perf_mode=DR,
                            )
            h1 = wp.tile([BC, HID], FP16, tag="l1_h")
            h1t = wp.tile([128, E, KH, BC], FP8, tag="l1_t")
            for h in range(2):
                sl = slice(h * HH, (h + 1) * HH)
                # blended bias last: += sum_e c[b,e] * (SZ1*b1[e, half])
                nc.tensor.matmul(
                    ps1h[h][:], c6[:], b1s[:, sl], start=False, stop=True
                )
                elu_half(ps1h[h], SZ1, h1, h * HH, f"l1{h}")
                tscale_half(h1, h1t, h, f"l1{h}")

            # ================= Layer 2 (DoubleRow fp8) =================
            ps2h = [
                pp.tile([BC, HH], FP32, name=f"l2ps{h}", tag="ps")
                for h in range(2)
            ]
            for h in range(2):
                sl = slice(h * HH, (h + 1) * HH)
                for g in range(2):
                    nc.tensor.matmul(
                        ps2h[h][:], zc3[:, g, :], w2zs[:, g, sl],
                        start=(g == 0), stop=False,
                    )
            h2 = wp.tile([BC, HID], FP16, tag="l2_h")
            h2t = wp.tile([128, E, KH, BC], FP8, tag="l2_t")
            # pair j contracts h1 columns [256j, 256j+256) = tscale half j
            for j in range(KH // 2):
                for e in range(E):
                    for h in range(2):
                        sl = slice(h * HH, (h + 1) * HH)
                        nc.tensor.matmul(
                            ps2h[h][:],
                            h1t[:, e, 2 * j : 2 * j + 2, :],
                            w2[:, e, 2 * j : 2 * j + 2, sl],
                            start=False,
                            stop=(e == E - 1 and j == KH // 2 - 1),
                            perf_mode=DR,
                        )
            for h in range(2):
                elu_half(ps2h[h], SZ2, h2, h * HH, f"l2{h}")
                tscale_half(h2, h2t, h, f"l2{h}")

            # ================= Layer 3 (DoubleRow fp8) =================
            res3 = wp.tile([BC, OUTP], FP32, tag="res3")
            ps3h = [
                pp.tile([BC, OH3], FP32, name=f"l3ps{h}", tag="ps")
                for h in range(2)
            ]
            for h in range(2):
                sl = slice(h * OH3, (h + 1) * OH3)
                for g in range(2):
                    nc.tensor.matmul(
                        ps3h[h][:], zc3[:, g, :], w3zs[:, g, sl],
                        start=(g == 0), stop=False,
                    )
            for j in range(KH // 2):
                for e in range(E):
                    for h in range(2):
                        sl = slice(h * OH3, (h + 1) * OH3)
                        nc.tensor.matmul(
                            ps3h[h][:],
                            h2t[:, e, 2 * j : 2 * j + 2, :],
                            w3[:, e, 2 * j : 2 * j + 2, sl],
                            start=False,
                            stop=(e == E - 1 and j == KH // 2 - 1),
                            perf_mode=DR,
                        )
            for h in range(2):
                sl = slice(h * OH3, (h + 1) * OH3)
                nc.vector.tensor_scalar(
                    res3[:, sl], ps3h[h][:], 1.0 / SZ2, None,
                    mybir.AluOpType.mult,
                )
                # stream each half out as soon as its copy lands
                eng = nc.scalar if h == 0 else nc.sync
                eng.dma_start(out=out_d[:, sl], in_=res3[:, sl])

    _split_waits(nc)
    _trim_tail(nc)
    return nc


def _trim_tail(nc):
    """Drop the second all-engine barrier round + sem-clear at the kernel
    tail: the first drain+barrier already guarantees completion, and the
    preamble re-initializes semaphores on any re-execution (verified by
    double-execution test)."""
    blk = nc.m.functions[0].blocks[-1]
    insts = blk.instructions
    cut = None
    for idx in range(len(insts) - 1, -1, -1):
        if type(insts[idx]).__name__ == "InstISA":
            cut = idx
            break
    if cut is not None:
        blk.instructions = insts[:cut]


_NC_CACHE = None


def _get_nc():
    global _NC_CACHE
    if _NC_CACHE is None:
        _NC_CACHE = build_nc()
    return _NC_CACHE


def _zgroup(wz, width):
    """[E, 33, width] (bias row + z rows, pre-scaled) -> [99, 2, width]
    where row 33e+r of group g holds expert 3g+e's row r."""
    t = wz.reshape(2, 3, ZR, width).transpose(1, 2, 0, 3)
    return np.ascontiguousarray(t.reshape(ZK, 2, width))


def make_in_maps(p_prev, blending_coef, z, w_l1, b_l1, w_l2, b_l2, w_l3, b_l3):
    f, h = np.float32, np.float16
    h0 = np.concatenate([z, p_prev], axis=1).astype(f)            # [B, IN]
    coef = blending_coef.astype(f)

    w1q = np.ascontiguousarray(                                    # [E,128,K1,HID]
        (SW * w_l1.astype(f)).astype(E4)
        .reshape(E, K1, 128, HID).transpose(0, 2, 1, 3)
    )
    b1s = (SZ1 * b_l1.astype(f)).astype(h)                         # [E, HID]
    w2z = np.concatenate(
        [b_l2.astype(f)[:, None, :], w_l2[:, :ZD, :].astype(f)], axis=1
    )                                                              # [E, 33, HID]
    w2zs = _zgroup((SZ2 * w2z).astype(h), HID)                     # [99, 2, HID]
    w2q = np.ascontiguousarray(                                    # [E,128,KH,HID]
        (SW * w_l2[:, ZD:, :].astype(f)).astype(E4)
        .reshape(E, KH, 128, HID).transpose(0, 2, 1, 3)
    )
    w3p = np.zeros((E, HID + ZD, OUTP), f)
    w3p[:, :, :OUT] = w_l3
    b3p = np.zeros((E, OUTP), f)
    b3p[:, :OUT] = b_l3
    w3z = np.concatenate([b3p[:, None, :], w3p[:, :ZD, :]], axis=1)
    w3zs = _zgroup((SZ2 * w3z).astype(h), OUTP)                    # [99, 2, OUTP]
    w3q = np.ascontiguousarray(                                    # [E,128,KH,OUTP]
        (SW * w3p[:, ZD:, :]).astype(E4)
        .reshape(E, KH, 128, OUTP).transpose(0, 2, 1, 3)
    )
    eye = np.eye(BC, dtype=f)

    in_maps = []
    for c in range(N_CORES):
        bs = slice(c * BC, (c + 1) * BC)
        cc = coef[bs]                                              # [BC, E]
        hv = h0[bs].T.reshape(K1, 128, BC)                         # [k, p, b]
        hc6 = np.zeros((128, E, K1P, BC), E4)                      # k13 stays 0
        hc6[:, :, :K1, :] = (
            SH0 * np.einsum("be,kpb->pekb", cc, hv)
        ).astype(E4)
        c6 = np.ascontiguousarray(cc.T).astype(h)                  # [E, BC]
        zext = np.concatenate(
            [np.ones((BC, 1), f), z[bs].astype(f)], axis=1
        ).T                                                        # [33, BC]
        zc3t = np.einsum("rb,bE->Erb", zext, cc)                   # [6, 33, BC]
        zc3 = np.ascontiguousarray(
            zc3t.reshape(2, 3, ZR, BC).transpose(1, 2, 0, 3).reshape(ZK, 2, BC)
        ).astype(h)
        dg6 = np.ascontiguousarray(                                # [BC, E*BC]
            np.einsum("be,bc->bec", SH * cc, eye).reshape(BC, E * BC)
        ).astype(h)
        in_maps.append(
            {
                "hc6": hc6, "c6": c6, "zc3": zc3, "dg6": dg6, "b1s": b1s,
                "w1q": w1q, "w2zs": w2zs, "w2q": w2q,
                "w3zs": w3zs, "w3q": w3q,
            }
        )
    return in_maps


def assemble_output(results):
    full = np.concatenate(
        [results[c]["outc"] for c in range(N_CORES)], axis=0
    )                                                              # [256, 640]
    return np.ascontiguousarray(full[:, :OUT]).astype(np.float32)


def kernel(p_prev, blending_coef, z, w_l1, b_l1, w_l2, b_l2, w_l3, b_l3):
    args = [
        np.asarray(a)
        for a in (p_prev, blending_coef, z, w_l1, b_l1, w_l2, b_l2, w_l3, b_l3)
    ]
    nc = _get_nc()
    in_maps = make_in_maps(*args)
    res = run_bass_kernel_spmd(nc, in_maps, CORE_IDS)
    return assemble_output(res.results)


# revision 31
# speedup vs baseline: 1.1210x; 1.0273x over previous
"""Trainium2 Bass kernel for nn_PredictionNet — data-parallel over batch.

Each of the 8 cores handles a 32-sample batch slice with ALL expert weights
resident in SBUF. No cross-core communication.

Two structural optimizations over the fp16 per-expert-blend version:

1. fp8 (e4m3) for every large weight tensor. The network re-injects the
   unit-scale z features at layers 2/3, so the hidden-path weights (w1, w2,
   w3 hidden rows) carry only ~0.4% of the output variance headroom and
   tolerate fp8 easily (measured 2.1e-3 absmax-rel vs f64). The z/bias rows
   (w2z, w3z) stay fp16. Weights are pre-scaled by 256 (acts by 16/64) on
   the host to live in the e4m3 normal range; descales fold into the ELU /
   final-copy scalar ops. Layer 1 runs DoubleRow fp8 matmuls (K=256 per
   instruction, 2x PE throughput).

2. Blend-free PSUM accumulation: the per-sample blend coefficient c[b,e]
   is folded into the *stationary* matmul operands (host-scaled hc6/zc3/c6;
   diag-scaled transpose for h1/h2), so the expert sum accumulates directly
   in PSUM. This removes all 24 per-expert DVE blend ops of the old design.
   The transpose+scale of h between layers is a single regular matmul per
   128-chunk against a [32, 6*32] block-of-diagonals operand.
"""

import sys

sys.path.insert(0, "/opt/trn_rl_repo")

import numpy as np
import ml_dtypes

import concourse.bass as bass
import concourse.mybir as mybir
import concourse.tile as tile
from concourse.bass_utils import run_bass_kernel_spmd

B, E = 256, 6
IN, HID, OUT, ZD = 1664, 512, 618, 32
N_CORES = 8
CORE_IDS = list(range(N_CORES))
BC = B // N_CORES         # 32 batch rows per core
K1 = IN // 128            # 13 real k-chunks, layer 1
K1P = 14                  # padded to 14 = 7 DoubleRow pairs (slot 13 zeroed)
KH = HID // 128           # 4 k-chunks for the hidden part of layers 2/3
OUTP = 640                # layer-3 output padded 618 -> 640
NH3 = 2                   # layer-3 output split into halves of 320 (psum bank)
OH3 = OUTP // NH3
ZR = 1 + ZD               # 33 rows: ones row (bias) + z
ZK = 3 * ZR               # 99: three experts' z-blocks stacked per group
FP32 = mybir.dt.float32
FP16 = mybir.dt.float16
FP8 = mybir.dt.float8e4
E4 = ml_dtypes.float8_e4m3
DR = mybir.MatmulPerfMode.DoubleRow

SH0 = 16.0                # h0 pre-scale into fp8
SW = 256.0                # weight pre-scale into fp8
SH = 64.0                 # hidden-act scale (folded into the diag operand)
SZ1 = SH0 * SW            # layer-1 psum scale
SZ2 = SH * SW             # layer-2/3 psum scale


def _split_waits(nc, max_waits=1):
    """neuronxcc walrus accepts only ONE sync-wait per instruction: hoist
    extras onto same-engine NoOps placed before the offending instruction."""
    n = 0
    for fn in nc.m.functions:
        for blk in fn.blocks:
            insts = blk.instructions
            if not any(
                i.sync_info is not None and len(i.sync_info.on_wait) > max_waits
                for i in insts
            ):
                continue
            out = []
            for inst in insts:
                si = inst.sync_info
                if si is not None and len(si.on_wait) > max_waits:
                    for w in si.on_wait[:-max_waits]:
                        n += 1
                        nop = mybir.InstNoOp(name=f"I-wfix{n}", ins=[], outs=[])
                        nop.engine = inst.engine
                        nop.sync_info = mybir.SyncInfo(on_wait=[w], on_update=[])
                        try:
                            nc.register_instruction(nop, overwrite=True)
                        except Exception:
                            pass
                        out.append(nop)
                    inst.sync_info = mybir.SyncInfo(
                        on_wait=list(si.on_wait[-max_waits:]),
                        on_update=list(si.on_update),
                    )
                out.append(inst)
            blk.instructions = out
    return n


def build_nc():
    nc = bass.Bass()

    hc6_d = nc.dram_tensor("hc6", [128, E, K1P, BC], FP8, kind="ExternalInput")
    c6_d = nc.dram_tensor("c6", [E, BC], FP16, kind="ExternalInput")
    zc3_d = nc.dram_tensor("zc3", [ZK, 2, BC], FP16, kind="ExternalInput")
    dg6_d = nc.dram_tensor("dg6", [BC, E * BC], FP16, kind="ExternalInput")
    b1s_d = nc.dram_tensor("b1s", [E, HID], FP16, kind="ExternalInput")
    w1q_d = nc.dram_tensor("w1q", [E, 128, K1, HID], FP8, kind="ExternalInput")
    w2zs_d = nc.dram_tensor("w2zs", [ZK, 2, HID], FP16, kind="ExternalInput")
    w2q_d = nc.dram_tensor("w2q", [E, 128, KH, HID], FP8, kind="ExternalInput")
    w3zs_d = nc.dram_tensor("w3zs", [ZK, 2, OUTP], FP16, kind="ExternalInput")
    w3q_d = nc.dram_tensor("w3q", [E, 128, KH, OUTP], FP8, kind="ExternalInput")
    out_d = nc.dram_tensor("outc", [BC, OUTP], FP32, kind="ExternalOutput")

    with tile.TileContext(nc) as tc:
        with (
            tc.tile_pool(name="const", bufs=1) as cp,
            tc.tile_pool(name="work", bufs=1) as wp,
            tc.tile_pool(name="psum", bufs=3, space="PSUM") as pp,
            tc.tile_pool(name="psumt", bufs=4, space="PSUM") as pt,
        ):
            # ---- DMA schedule. Critical-path-first on the two fast HWDGE
            # queues: layer-1 operands (hc6 on sync; c6/b1s on scalar) ahead
            # of the weight slabs; w1 expert halves interleaved across both
            # queues in PE-consumption order. Slack tensors ride SWDGE.
            hc6 = cp.tile([128, E, K1P, BC], FP8)
            nc.sync.dma_start(out=hc6[:], in_=hc6_d[:])
            c6 = cp.tile([E, BC], FP16)
            nc.scalar.dma_start(out=c6[:], in_=c6_d[:])
            b1s = cp.tile([E, HID], FP16)
            nc.scalar.dma_start(out=b1s[:], in_=b1s_d[:])
            # w1 expert slabs split in k-halves in PE-consumption order:
            # a-halves (k0..7, 0.52MB) on sync, b-halves (k8..12, 0.33MB)
            # on scalar. The scalar queue measurably starts moving ~4us
            # after sync, so the ~3.5/2.0 MB byte split makes both queues
            # finish w1 near-simultaneously (~23us).
            w1 = cp.tile([128, E, K1P, HID], FP8)
            # zero the padded 14th k-slot once; DoubleRow pair 6 reads it
            nc.vector.memset(w1[:, :, K1P - 1, :], 0.0)
            KA = 8
            for e in range(E):
                nc.sync.dma_start(
                    out=w1[:, e, :KA, :], in_=w1q_d[e, :, :KA, :]
                )
                nc.scalar.dma_start(
                    out=w1[:, e, KA:K1, :], in_=w1q_d[e, :, KA:, :]
                )
            # ---- slack tensors (needed only from layer 2 on) ----
            zc3 = cp.tile([ZK, 2, BC], FP16)
            nc.gpsimd.dma_start(out=zc3[:], in_=zc3_d[:])
            dg6 = cp.tile([BC, E * BC], FP16)
            nc.gpsimd.dma_start(out=dg6[:], in_=dg6_d[:])
            w2zs = cp.tile([ZK, 2, HID], FP16)
            nc.gpsimd.dma_start(out=w2zs[:], in_=w2zs_d[:])
            w3zs = cp.tile([ZK, 2, OUTP], FP16)
            nc.gpsimd.dma_start(out=w3zs[:], in_=w3zs_d[:])
            w2 = cp.tile([128, E, KH, HID], FP8)
            for e in range(E):
                eng = nc.sync if e % 2 == 0 else nc.scalar
                eng.dma_start(out=w2[:, e, :, :], in_=w2q_d[e])
            w3 = cp.tile([128, E, KH, OUTP], FP8)
            for e in range(E):
                eng = nc.sync if e % 2 == 0 else nc.scalar
                eng.dma_start(out=w3[:, e, :, :], in_=w3q_d[e])

            HH = HID // 2     # 256-column half of a hidden layer

            def elu_half(ps, s, h, c0, tag):
                """h[:, c0:c0+HH] = ELU(ps / s) in fp16 via
                relu(x) + min(exp(x)-1, 0). exp/relu read PSUM directly on
                ACT (descale fused via scale=); x is O(1) so exp is safe."""
                texp = wp.tile([BC, HH], FP16, tag=f"{tag}_exp")
                nc.scalar.activation(
                    texp[:], ps[:], mybir.ActivationFunctionType.Exp,
                    scale=1.0 / s,
                )
                trel = wp.tile([BC, HH], FP16, tag=f"{tag}_rel")
                nc.scalar.activation(
                    trel[:], ps[:], mybir.ActivationFunctionType.Relu,
                    scale=1.0 / s,
                )
                tmin = wp.tile([BC, HH], FP16, tag=f"{tag}_min")
                nc.vector.tensor_scalar(
                    tmin[:], texp[:], -1.0, 0.0,
                    mybir.AluOpType.add, mybir.AluOpType.min,
                )
                nc.vector.tensor_tensor(
                    h[:, c0 : c0 + HH], tmin[:], trel[:], mybir.AluOpType.add
                )

            def tscale_half(h, ht, half, tag):
                """ht[p, e, j, b] = 64 * c[b,e] * h[b, 128j+p] for the two
                128-chunks j of column-half `half`. One regular matmul per
                chunk against the block-of-diags dg6 [32, 6*32] does
                transpose + per-expert coef scaling; PSUM->SBUF fp8 casts
                ride the ACT engine (DVE is on ELU duty)."""
                for j in range(2 * half, 2 * half + 2):
                    ps = pt.tile([128, E, BC], FP32, name=f"{tag}_tp{j}", tag="tpose")
                    nc.tensor.matmul(
                        ps[:], h[:, j * 128 : (j + 1) * 128], dg6[:],
                        start=True, stop=True,
                    )
                    nc.scalar.activation(
                        ht[:, :, j, :], ps[:], mybir.ActivationFunctionType.Copy
                    )

            # All three layers are split into two output-column half-chains
            # (A = first half, B = second half) so the ELU + transpose of
            # half A overlaps PE work of half B — the PE stream never has a
            # multi-us bubble and the gated clock stays at 2.4 GHz.

            # ================= Layer 1 (DoubleRow fp8, 7 pairs) =============
            ps1h = [
                pp.tile([BC, HH], FP32, name=f"l1ps{h}", tag="ps")
                for h in range(2)
            ]
            # Consume w1 in DMA-arrival order: a-halves (pairs 0..3) stream
            # on sync from the start, b-halves (pairs 4..6) on the ~4us-late
            # scalar queue — so a-parts lead by two experts and b-parts
            # trail, keeping PE fed from both queues.
            SA, SB = (0, KA // 2), (KA // 2, K1P // 2)
            L1_ORDER = [
                (0, SA), (1, SA), (0, SB), (2, SA), (1, SB), (3, SA),
                (2, SB), (4, SA), (3, SB), (5, SA), (4, SB), (5, SB),
            ]
            for (e, (j0, j1)) in L1_ORDER:
                for h in range(2):
                    sl = slice(h * HH, (h + 1) * HH)
                    for j in range(j0, j1):
                        nc.tensor.matmul(
                            ps1h[h][:],
                            hc6[:, e, 2 * j : 2 * j + 2, :],
                            w1[:, e, 2 * j : 2 * j + 2, sl],
                            start=(e == 0 and j == 0), stop=False,
                            perf_mode=DR,
                        )
            h1 = wp.tile([BC, HID], FP16, tag="l1_h")
            h1t = wp.tile([128, E, KH, BC], FP8, tag="l1_t")
            for h in range(2):
                sl = slice(h * HH, (h + 1) * HH)
                # blended bias last: += sum_e c[b,e] * (SZ1*b1[e, half])
                nc.tensor.matmul(
                    ps1h[h][:], c6[:], b1s[:, sl], start=False, stop=True
                )
                elu_half(ps1h[h], SZ1, h1, h * HH, f"l1{h}")
                tscale_half(h1, h1t, h, f"l1{h}")

            # ================= Layer 2 (DoubleRow fp8) =================
            ps2h = [
                pp.tile([BC, HH], FP32, name=f"l2ps{h}", tag="ps")
                for h in range(2)
            ]
            for h in range(2):
                sl = slice(h * HH, (h + 1) * HH)
                for g in range(2):
                    nc.tensor.matmul(
                        ps2h[h][:], zc3[:, g, :], w2zs[:, g, sl],
                        start=(g == 0), stop=False,
                    )
            h2 = wp.tile([BC, HID], FP16, tag="l2_h")
            h2t = wp.tile([128, E, KH, BC], FP8, tag="l2_t")
            # pair j contracts h1 columns [256j, 256j+256) = tscale half j.
            # Expert order [0,2,4,1,3,5]: even slabs ride the early sync
            # queue, odd slabs the late scalar queue.
            E_ORDER = [0, 2, 4, 1, 3, 5]
            for j in range(KH // 2):
                for ei, e in enumerate(E_ORDER):
                    for h in range(2):
                        sl = slice(h * HH, (h + 1) * HH)
                        nc.tensor.matmul(
                            ps2h[h][:],
                            h1t[:, e, 2 * j : 2 * j + 2, :],
                            w2[:, e, 2 * j : 2 * j + 2, sl],
                            start=False,
                            stop=(ei == E - 1 and j == KH // 2 - 1),
                            perf_mode=DR,
                        )
            for h in range(2):
                elu_half(ps2h[h], SZ2, h2, h * HH, f"l2{h}")
                tscale_half(h2, h2t, h, f"l2{h}")

            # ================= Layer 3 (DoubleRow fp8) =================
            res3 = wp.tile([BC, OUTP], FP32, tag="res3")
            ps3h = [
                pp.tile([BC, OH3], FP32, name=f"l3ps{h}", tag="ps")
                for h in range(2)
            ]
            for h in range(2):
                sl = slice(h * OH3, (h + 1) * OH3)
                for g in range(2):
                    nc.tensor.matmul(
                        ps3h[h][:], zc3[:, g, :], w3zs[:, g, sl],
                        start=(g == 0), stop=False,
                    )
            for j in range(KH // 2):
                for ei, e in enumerate(E_ORDER):
                    for h in range(2):
                        sl = slice(h * OH3, (h + 1) * OH3)
                        nc.tensor.matmul(
                            ps3h[h][:],
                            h2t[:, e, 2 * j : 2 * j + 2, :],
                            w3[:, e, 2 * j : 2 * j + 2, sl],
                            start=False,
                            stop=(ei == E - 1 and j == KH // 2 - 1),
                            perf_mode=DR,
                        )
            for h in range(2):
                sl = slice(h * OH3, (h + 1) * OH3)
                nc.vector.tensor_scalar(
                    res3[:, sl], ps3h[h][:], 1.0 / SZ2, None,
                    mybir.AluOpType.mult,
                )
                # stream each half out as soon as its copy lands
                eng = nc.scalar if h == 0 else nc.sync
                eng.dma_start(out=out_d[:, sl], in_=res3[:, sl])

    _split_waits(nc)
    _trim_tail(nc)
    return nc


def _trim_tail(nc):
    """Drop the second all-engine barrier round + sem-clear at the kernel
    tail: the first drain+barrier already guarantees completion, and the
    preamble re-initializes semaphores on any re-execution (verified by
    double-execution test)."""
    blk = nc.m.functions[0].blocks[-1]
    insts = blk.instructions
    cut = None
    for idx in range(len(insts) - 1, -1, -1):
        if type(insts[idx]).__name__ == "InstISA":
            cut = idx
            break
    if cut is not None:
        blk.instructions = insts[:cut]


_NC_CACHE = None


def _get_nc():
    global _NC_CACHE
    if _NC_CACHE is None:
        _NC_CACHE = build_nc()
    return _NC_CACHE


def _zgroup(wz, width):
    """[E, 33, width] (bias row + z rows, pre-scaled) -> [99, 2, width]
    where row 33e+r of group g holds expert 3g+e's row r."""
    t = wz.reshape(2, 3, ZR, width).transpose(1, 2, 0, 3)
    return np.ascontiguousarray(t.reshape(ZK, 2, width))


def make_in_maps(p_prev, blending_coef, z, w_l1, b_l1, w_l2, b_l2, w_l3, b_l3):
    f, h = np.float32, np.float16
    h0 = np.concatenate([z, p_prev], axis=1).astype(f)            # [B, IN]
    coef = blending_coef.astype(f)

    w1q = np.ascontiguousarray(                                    # [E,128,K1,HID]
        (SW * w_l1.astype(f)).astype(E4)
        .reshape(E, K1, 128, HID).transpose(0, 2, 1, 3)
    )
    b1s = (SZ1 * b_l1.astype(f)).astype(h)                         # [E, HID]
    w2z = np.concatenate(
        [b_l2.astype(f)[:, None, :], w_l2[:, :ZD, :].astype(f)], axis=1
    )                                                              # [E, 33, HID]
    w2zs = _zgroup((SZ2 * w2z).astype(h), HID)                     # [99, 2, HID]
    w2q = np.ascontiguousarray(                                    # [E,128,KH,HID]
        (SW * w_l2[:, ZD:, :].astype(f)).astype(E4)
        .reshape(E, KH, 128, HID).transpose(0, 2, 1, 3)
    )
    w3p = np.zeros((E, HID + ZD, OUTP), f)
    w3p[:, :, :OUT] = w_l3
    b3p = np.zeros((E, OUTP), f)
    b3p[:, :OUT] = b_l3
    w3z = np.concatenate([b3p[:, None, :], w3p[:, :ZD, :]], axis=1)
    w3zs = _zgroup((SZ2 * w3z).astype(h), OUTP)                    # [99, 2, OUTP]
    w3q = np.ascontiguousarray(                                    # [E,128,KH,OUTP]
        (SW * w3p[:, ZD:, :]).astype(E4)
        .reshape(E, KH, 128, OUTP).transpose(0, 2, 1, 3)
    )
    eye = np.eye(BC, dtype=f)

    in_maps = []
    for c in range(N_CORES):
        bs = slice(c * BC, (c + 1) * BC)
        cc = coef[bs]                                              # [BC, E]
        hv = h0[bs].T.reshape(K1, 128, BC)                         # [k, p, b]
        hc6 = np.zeros((128, E, K1P, BC), E4)                      # k13 stays 0
        hc6[:, :, :K1, :] = (
            SH0 * np.einsum("be,kpb->pekb", cc, hv)
        ).astype(E4)
        c6 = np.ascontiguousarray(cc.T).astype(h)                  # [E, BC]
        zext = np.concatenate(
            [np.ones((BC, 1), f), z[bs].astype(f)], axis=1
        ).T                                                        # [33, BC]
        zc3t = np.einsum("rb,bE->Erb", zext, cc)                   # [6, 33, BC]
        zc3 = np.ascontiguousarray(
            zc3t.reshape(2, 3, ZR, BC).transpose(1, 2, 0, 3).reshape(ZK, 2, BC)
        ).astype(h)
        dg6 = np.ascontiguousarray(                                # [BC, E*BC]
            np.einsum("be,bc->bec", SH * cc, eye).reshape(BC, E * BC)
        ).astype(h)
        in_maps.append(
            {
                "hc6": hc6, "c6": c6, "zc3": zc3, "dg6": dg6, "b1s": b1s,
                "w1q": w1q, "w2zs": w2zs, "w2q": w2q,
                "w3zs": w3zs, "w3q": w3q,
            }
        )
    return in_maps


def assemble_output(results):
    full = np.concatenate(
        [results[c]["outc"] for c in range(N_CORES)], axis=0
    )                                                              # [256, 640]
    return np.ascontiguousarray(full[:, :OUT]).astype(np.float32)


def kernel(p_prev, blending_coef, z, w_l1, b_l1, w_l2, b_l2, w_l3, b_l3):
    args = [
        np.asarray(a)
        for a in (p_prev, blending_coef, z, w_l1, b_l1, w_l2, b_l2, w_l3, b_l3)
    ]
    nc = _get_nc()
    in_maps = make_in_maps(*args)
    res = run_bass_kernel_spmd(nc, in_maps, CORE_IDS)
    return assemble_output(res.results)


# revision 33
# speedup vs baseline: 1.1640x; 1.0384x over previous
"""Trainium2 Bass kernel for nn_PredictionNet — data-parallel over batch.

Each of the 8 cores handles a 32-sample batch slice with ALL expert weights
resident in SBUF. No cross-core communication.

Two structural optimizations over the fp16 per-expert-blend version:

1. fp8 (e4m3) for every large weight tensor. The network re-injects the
   unit-scale z features at layers 2/3, so the hidden-path weights (w1, w2,
   w3 hidden rows) carry only ~0.4% of the output variance headroom and
   tolerate fp8 easily (measured 2.1e-3 absmax-rel vs f64). The z/bias rows
   (w2z, w3z) stay fp16. Weights are pre-scaled by 256 (acts by 16/64) on
   the host to live in the e4m3 normal range; descales fold into the ELU /
   final-copy scalar ops. Layer 1 runs DoubleRow fp8 matmuls (K=256 per
   instruction, 2x PE throughput).

2. Blend-free PSUM accumulation: the per-sample blend coefficient c[b,e]
   is folded into the *stationary* matmul operands (host-scaled hc6/zc3/c6;
   diag-scaled transpose for h1/h2), so the expert sum accumulates directly
   in PSUM. This removes all 24 per-expert DVE blend ops of the old design.
   The transpose+scale of h between layers is a single regular matmul per
   128-chunk against a [32, 6*32] block-of-diagonals operand.
"""

import sys

sys.path.insert(0, "/opt/trn_rl_repo")

import numpy as np
import ml_dtypes

import concourse.bass as bass
import concourse.mybir as mybir
import concourse.tile as tile
from concourse.bass_utils import run_bass_kernel_spmd

B, E = 256, 6
IN, HID, OUT, ZD = 1664, 512, 618, 32
N_CORES = 8
CORE_IDS = list(range(N_CORES))
BC = B // N_CORES         # 32 batch rows per core
K1 = IN // 128            # 13 real k-chunks, layer 1
K1P = 14                  # padded to 14 = 7 DoubleRow pairs (slot 13 zeroed)
KH = HID // 128           # 4 k-chunks for the hidden part of layers 2/3
OUTP = 640                # layer-3 output padded 618 -> 640
NH3 = 2                   # layer-3 output split into halves of 320 (psum bank)
OH3 = OUTP // NH3
ZR = 1 + ZD               # 33 rows: ones row (bias) + z
ZK = 3 * ZR               # 99: three experts' z-blocks stacked per group
FP32 = mybir.dt.float32
FP16 = mybir.dt.float16
FP8 = mybir.dt.float8e4
E4 = ml_dtypes.float8_e4m3
DR = mybir.MatmulPerfMode.DoubleRow

SH0 = 16.0                # h0 pre-scale into fp8
SW = 256.0                # weight pre-scale into fp8
SH = 64.0                 # hidden-act scale (folded into the diag operand)
SZ1 = SH0 * SW            # layer-1 psum scale
SZ2 = SH * SW             # layer-2/3 psum scale


def _split_waits(nc, max_waits=1):
    """neuronxcc walrus accepts only ONE sync-wait per instruction: hoist
    extras onto same-engine NoOps placed before the offending instruction."""
    n = 0
    for fn in nc.m.functions:
        for blk in fn.blocks:
            insts = blk.instructions
            if not any(
                i.sync_info is not None and len(i.sync_info.on_wait) > max_waits
                for i in insts
            ):
                continue
            out = []
            for inst in insts:
                si = inst.sync_info
                if si is not None and len(si.on_wait) > max_waits:
                    for w in si.on_wait[:-max_waits]:
                        n += 1
                        nop = mybir.InstNoOp(name=f"I-wfix{n}", ins=[], outs=[])
                        nop.engine = inst.engine
                        nop.sync_info = mybir.SyncInfo(on_wait=[w], on_update=[])
                        try:
                            nc.register_instruction(nop, overwrite=True)
                        except Exception:
                            pass
                        out.append(nop)
                    inst.sync_info = mybir.SyncInfo(
                        on_wait=list(si.on_wait[-max_waits:]),
                        on_update=list(si.on_update),
                    )
                out.append(inst)
            blk.instructions = out
    return n


def build_nc():
    nc = bass.Bass()

    hc6_d = nc.dram_tensor("hc6", [128, E, K1P, BC], FP8, kind="ExternalInput")
    c6_d = nc.dram_tensor("c6", [E, BC], FP16, kind="ExternalInput")
    zc3_d = nc.dram_tensor("zc3", [ZK, 2, BC], FP16, kind="ExternalInput")
    dg6_d = nc.dram_tensor("dg6", [BC, E * BC], FP16, kind="ExternalInput")
    b1s_d = nc.dram_tensor("b1s", [E, HID], FP16, kind="ExternalInput")
    w1q_d = nc.dram_tensor("w1q", [E, 128, K1, HID], FP8, kind="ExternalInput")
    w2zs_d = nc.dram_tensor("w2zs", [ZK, 2, HID], FP16, kind="ExternalInput")
    w2q_d = nc.dram_tensor("w2q", [E, 128, KH, HID], FP8, kind="ExternalInput")
    w3zs_d = nc.dram_tensor("w3zs", [ZK, 2, OUTP], FP16, kind="ExternalInput")
    w3q_d = nc.dram_tensor("w3q", [E, 128, KH, OUTP], FP8, kind="ExternalInput")
    out_d = nc.dram_tensor("outc", [BC, OUTP], FP32, kind="ExternalOutput")

    with tile.TileContext(nc) as tc:
        with (
            tc.tile_pool(name="const", bufs=1) as cp,
            tc.tile_pool(name="work", bufs=1) as wp,
            tc.tile_pool(name="psum", bufs=2, space="PSUM") as pp,
            tc.tile_pool(name="psumt", bufs=4, space="PSUM") as pt,
        ):
            # ---- DMA schedule. Critical-path-first on the two fast HWDGE
            # queues: layer-1 operands (hc6 on sync; c6/b1s on scalar) ahead
            # of the weight slabs; w1 expert halves interleaved across both
            # queues in PE-consumption order. Slack tensors ride SWDGE.
            hc6 = cp.tile([128, E, K1P, BC], FP8)
            nc.sync.dma_start(out=hc6[:], in_=hc6_d[:])
            c6 = cp.tile([E, BC], FP16)
            nc.scalar.dma_start(out=c6[:], in_=c6_d[:])
            b1s = cp.tile([E, HID], FP16)
            nc.scalar.dma_start(out=b1s[:], in_=b1s_d[:])
            # w1 expert slabs split in k-halves in PE-consumption order:
            # a-halves (k0..7, 0.52MB) on sync, b-halves (k8..12, 0.33MB)
            # on scalar. The scalar queue measurably starts moving ~4us
            # after sync, so the ~3.5/2.0 MB byte split makes both queues
            # finish w1 near-simultaneously (~23us).
            w1 = cp.tile([128, E, K1P, HID], FP8)
            # zero the padded 14th k-slot once; DoubleRow pair 6 reads it
            nc.vector.memset(w1[:, :, K1P - 1, :], 0.0)
            KA = 8
            for e in range(E):
                nc.sync.dma_start(
                    out=w1[:, e, :KA, :], in_=w1q_d[e, :, :KA, :]
                )
                nc.scalar.dma_start(
                    out=w1[:, e, KA:K1, :], in_=w1q_d[e, :, KA:, :]
                )
            # ---- slack tensors (needed only from layer 2 on); the z-path
            # weights go first so the scheduler-hoisted z matmuls never
            # stall the in-order PE stream ----
            zc3 = cp.tile([ZK, 2, BC], FP16)
            nc.gpsimd.dma_start(out=zc3[:], in_=zc3_d[:])
            w2zs = cp.tile([ZK, 2, HID], FP16)
            nc.gpsimd.dma_start(out=w2zs[:], in_=w2zs_d[:])
            w3zs = cp.tile([ZK, 2, OUTP], FP16)
            nc.gpsimd.dma_start(out=w3zs[:], in_=w3zs_d[:])
            dg6 = cp.tile([BC, E * BC], FP16)
            nc.gpsimd.dma_start(out=dg6[:], in_=dg6_d[:])
            w2 = cp.tile([128, E, KH, HID], FP8)
            for e in range(E):
                eng = nc.sync if e % 2 == 0 else nc.scalar
                eng.dma_start(out=w2[:, e, :, :], in_=w2q_d[e])
            w3 = cp.tile([128, E, KH, OUTP], FP8)
            for e in range(E):
                eng = nc.sync if e % 2 == 0 else nc.scalar
                eng.dma_start(out=w3[:, e, :, :], in_=w3q_d[e])

            HH = HID // 2     # 256-column half of a hidden layer

            def elu_half(ps, s, h, c0, tag):
                """h[:, c0:c0+HH] = ELU(ps / s) in fp16 via
                relu(x) + min(exp(x)-1, 0). exp/relu read PSUM directly on
                ACT (descale fused via scale=); x is O(1) so exp is safe."""
                texp = wp.tile([BC, HH], FP16, tag=f"{tag}_exp")
                nc.scalar.activation(
                    texp[:], ps[:], mybir.ActivationFunctionType.Exp,
                    scale=1.0 / s,
                )
                trel = wp.tile([BC, HH], FP16, tag=f"{tag}_rel")
                nc.scalar.activation(
                    trel[:], ps[:], mybir.ActivationFunctionType.Relu,
                    scale=1.0 / s,
                )
                tmin = wp.tile([BC, HH], FP16, tag=f"{tag}_min")
                nc.vector.tensor_scalar(
                    tmin[:], texp[:], -1.0, 0.0,
                    mybir.AluOpType.add, mybir.AluOpType.min,
                )
                nc.vector.tensor_tensor(
                    h[:, c0 : c0 + HH], tmin[:], trel[:], mybir.AluOpType.add
                )

            def tscale_half(h, ht, half, tag):
                """ht[p, e, j, b] = 64 * c[b,e] * h[b, 128j+p] for the two
                128-chunks j of column-half `half`. One regular matmul per
                chunk against the block-of-diags dg6 [32, 6*32] does
                transpose + per-expert coef scaling; PSUM->SBUF fp8 casts
                ride the ACT engine (DVE is on ELU duty)."""
                for j in range(2 * half, 2 * half + 2):
                    ps = pt.tile([128, E, BC], FP32, name=f"{tag}_tp{j}", tag="tpose")
                    nc.tensor.matmul(
                        ps[:], h[:, j * 128 : (j + 1) * 128], dg6[:],
                        start=True, stop=True,
                    )
                    nc.scalar.activation(
                        ht[:, :, j, :], ps[:], mybir.ActivationFunctionType.Copy
                    )

            # All three layers are split into two output-column half-chains
            # (A = first half, B = second half) so the ELU + transpose of
            # half A overlaps PE work of half B — the PE stream never has a
            # multi-us bubble and the gated clock stays at 2.4 GHz.

            # ================= Layer 1 (DoubleRow fp8, 7 pairs) =============
            ps1h = [
                pp.tile([BC, HH], FP32, name=f"l1ps{h}", tag="ps")
                for h in range(2)
            ]
            # Consume w1 in DMA-arrival order: a-halves (pairs 0..3) stream
            # on sync from the start, b-halves (pairs 4..6) on the ~4us-late
            # scalar queue — so a-parts lead by two experts and b-parts
            # trail, keeping PE fed from both queues.
            SA, SB = (0, KA // 2), (KA // 2, K1P // 2)
            L1_ORDER = [
                (0, SA), (1, SA), (0, SB), (2, SA), (1, SB), (3, SA),
                (2, SB), (4, SA), (3, SB), (5, SA), (4, SB), (5, SB),
            ]
            for (e, (j0, j1)) in L1_ORDER:
                for h in range(2):
                    sl = slice(h * HH, (h + 1) * HH)
                    for j in range(j0, j1):
                        nc.tensor.matmul(
                            ps1h[h][:],
                            hc6[:, e, 2 * j : 2 * j + 2, :],
                            w1[:, e, 2 * j : 2 * j + 2, sl],
                            start=(e == 0 and j == 0), stop=False,
                            perf_mode=DR,
                        )
            h1 = wp.tile([BC, HID], FP16, tag="l1_h")
            h1t = wp.tile([128, E, KH, BC], FP8, tag="l1_t")
            for h in range(2):
                sl = slice(h * HH, (h + 1) * HH)
                # blended bias last: += sum_e c[b,e] * (SZ1*b1[e, half])
                nc.tensor.matmul(
                    ps1h[h][:], c6[:], b1s[:, sl], start=False, stop=True
                )
                elu_half(ps1h[h], SZ1, h1, h * HH, f"l1{h}")
                tscale_half(h1, h1t, h, f"l1{h}")

            # ================= Layer 2 (DoubleRow fp8) =================
            ps2h = [
                pp.tile([BC, HH], FP32, name=f"l2ps{h}", tag="ps")
                for h in range(2)
            ]
            for h in range(2):
                sl = slice(h * HH, (h + 1) * HH)
                for g in range(2):
                    nc.tensor.matmul(
                        ps2h[h][:], zc3[:, g, :], w2zs[:, g, sl],
                        start=(g == 0), stop=False,
                    )
            h2 = wp.tile([BC, HID], FP16, tag="l2_h")
            h2t = wp.tile([128, E, KH, BC], FP8, tag="l2_t")
            # pair j contracts h1 columns [256j, 256j+256) = tscale half j.
            # Expert order [0,2,4,1,3,5]: even slabs ride the early sync
            # queue, odd slabs the late scalar queue.
            E_ORDER = [0, 2, 4, 1, 3, 5]
            for j in range(KH // 2):
                for ei, e in enumerate(E_ORDER):
                    for h in range(2):
                        sl = slice(h * HH, (h + 1) * HH)
                        nc.tensor.matmul(
                            ps2h[h][:],
                            h1t[:, e, 2 * j : 2 * j + 2, :],
                            w2[:, e, 2 * j : 2 * j + 2, sl],
                            start=False,
                            stop=(ei == E - 1 and j == KH // 2 - 1),
                            perf_mode=DR,
                        )
            for h in range(2):
                elu_half(ps2h[h], SZ2, h2, h * HH, f"l2{h}")
                tscale_half(h2, h2t, h, f"l2{h}")

            # ================= Layer 3 (DoubleRow fp8) =================
            res3 = wp.tile([BC, OUTP], FP32, tag="res3")
            ps3h = [
                pp.tile([BC, OH3], FP32, name=f"l3ps{h}", tag="ps")
                for h in range(2)
            ]
            for h in range(2):
                sl = slice(h * OH3, (h + 1) * OH3)
                for g in range(2):
                    nc.tensor.matmul(
                        ps3h[h][:], zc3[:, g, :], w3zs[:, g, sl],
                        start=(g == 0), stop=False,
                    )
            for j in range(KH // 2):
                for ei, e in enumerate(E_ORDER):
                    for h in range(2):
                        sl = slice(h * OH3, (h + 1) * OH3)
                        nc.tensor.matmul(
                            ps3h[h][:],
                            h2t[:, e, 2 * j : 2 * j + 2, :],
                            w3[:, e, 2 * j : 2 * j + 2, sl],
                            start=False,
                            stop=(ei == E - 1 and j == KH // 2 - 1),
                            perf_mode=DR,
                        )
            for h in range(2):
                sl = slice(h * OH3, (h + 1) * OH3)
                nc.vector.tensor_scalar(
                    res3[:, sl], ps3h[h][:], 1.0 / SZ2, None,
                    mybir.AluOpType.mult,
                )
                # stream each half out as soon as its copy lands
                eng = nc.scalar if h == 0 else nc.sync
                eng.dma_start(out=out_d[:, sl], in_=res3[:, sl])

    _split_waits(nc)
    _trim_tail(nc)
    return nc


def _trim_tail(nc):
    """Drop the second all-engine barrier round + sem-clear at the kernel
    tail: the first drain+barrier already guarantees completion, and the
    preamble re-initializes semaphores on any re-execution (verified by
    double-execution test)."""
    blk = nc.m.functions[0].blocks[-1]
    insts = blk.instructions
    cut = None
    for idx in range(len(insts) - 1, -1, -1):
        if type(insts[idx]).__name__ == "InstISA":
            cut = idx
            break
    if cut is not None:
        blk.instructions = insts[:cut]


_NC_CACHE = None


def _get_nc():
    global _NC_CACHE
    if _NC_CACHE is None:
        _NC_CACHE = build_nc()
    return _NC_CACHE


def _zgroup(wz, width):
    """[E, 33, width] (bias row + z rows, pre-scaled) -> [99, 2, width]
    where row 33e+r of group g holds expert 3g+e's row r."""
    t = wz.reshape(2, 3, ZR, width).transpose(1, 2, 0, 3)
    return np.ascontiguousarray(t.reshape(ZK, 2, width))


def make_in_maps(p_prev, blending_coef, z, w_l1, b_l1, w_l2, b_l2, w_l3, b_l3):
    f, h = np.float32, np.float16
    h0 = np.concatenate([z, p_prev], axis=1).astype(f)            # [B, IN]
    coef = blending_coef.astype(f)

    w1q = np.ascontiguousarray(                                    # [E,128,K1,HID]
        (SW * w_l1.astype(f)).astype(E4)
        .reshape(E, K1, 128, HID).transpose(0, 2, 1, 3)
    )
    b1s = (SZ1 * b_l1.astype(f)).astype(h)                         # [E, HID]
    w2z = np.concatenate(
        [b_l2.astype(f)[:, None, :], w_l2[:, :ZD, :].astype(f)], axis=1
    )                                                              # [E, 33, HID]
    w2zs = _zgroup((SZ2 * w2z).astype(h), HID)                     # [99, 2, HID]
    w2q = np.ascontiguousarray(                                    # [E,128,KH,HID]
        (SW * w_l2[:, ZD:, :].astype(f)).astype(E4)
        .reshape(E, KH, 128, HID).transpose(0, 2, 1, 3)
    )
    w3p = np.zeros((E, HID + ZD, OUTP), f)
    w3p[:, :, :OUT] = w_l3
    b3p = np.zeros((E, OUTP), f)
    b3p[:, :OUT] = b_l3
    w3z = np.concatenate([b3p[:, None, :], w3p[:, :ZD, :]], axis=1)
    w3zs = _zgroup((SZ2 * w3z).astype(h), OUTP)                    # [99, 2, OUTP]
    w3q = np.ascontiguousarray(                                    # [E,128,KH,OUTP]
        (SW * w3p[:, ZD:, :]).astype(E4)
        .reshape(E, KH, 128, OUTP).transpose(0, 2, 1, 3)
    )
    eye = np.eye(BC, dtype=f)

    in_maps = []
    for c in range(N_CORES):
        bs = slice(c * BC, (c + 1) * BC)
        cc = coef[bs]                                              # [BC, E]
        hv = h0[bs].T.reshape(K1, 128, BC)                         # [k, p, b]
        hc6 = np.zeros((128, E, K1P, BC), E4)                      # k13 stays 0
        hc6[:, :, :K1, :] = (
            SH0 * np.einsum("be,kpb->pekb", cc, hv)
        ).astype(E4)
        c6 = np.ascontiguousarray(cc.T).astype(h)                  # [E, BC]
        zext = np.concatenate(
            [np.ones((BC, 1), f), z[bs].astype(f)], axis=1
        ).T                                                        # [33, BC]
        zc3t = np.einsum("rb,bE->Erb", zext, cc)                   # [6, 33, BC]
        zc3 = np.ascontiguousarray(
            zc3t.reshape(2, 3, ZR, BC).transpose(1, 2, 0, 3).reshape(ZK, 2, BC)
        ).astype(h)
        dg6 = np.ascontiguousarray(                                # [BC, E*BC]
            np.einsum("be,bc->bec", SH * cc, eye).reshape(BC, E * BC)
        ).astype(h)
        in_maps.append(
            {
                "hc6": hc6, "c6": c6, "zc3": zc3, "dg6": dg6, "b1s": b1s,
                "w1q": w1q, "w2zs": w2zs, "w2q": w2q,
                "w3zs": w3zs, "w3q": w3q,
            }
        )
    return in_maps


def assemble_output(results):
    full = np.concatenate(
        [results[c]["outc"] for c in range(N_CORES)], axis=0
    )                                                              # [256, 640]
    return np.ascontiguousarray(full[:, :OUT]).astype(np.float32)


def kernel(p_prev, blending_coef, z, w_l1, b_l1, w_l2, b_l2, w_l3, b_l3):
    args = [
        np.asarray(a)
        for a in (p_prev, blending_coef, z, w_l1, b_l1, w_l2, b_l2, w_l3, b_l3)
    ]
    nc = _get_nc()
    in_maps = make_in_maps(*args)
    res = run_bass_kernel_spmd(nc, in_maps, CORE_IDS)
    return assemble_output(res.results)


# revision 36
# speedup vs baseline: 1.1665x; 1.0022x over previous
"""Trainium2 Bass kernel for nn_PredictionNet — data-parallel over batch.

Each of the 8 cores handles a 32-sample batch slice with ALL expert weights
resident in SBUF. No cross-core communication.

Two structural optimizations over the fp16 per-expert-blend version:

1. fp8 (e4m3) for every large weight tensor. The network re-injects the
   unit-scale z features at layers 2/3, so the hidden-path weights (w1, w2,
   w3 hidden rows) carry only ~0.4% of the output variance headroom and
   tolerate fp8 easily (measured 2.1e-3 absmax-rel vs f64). The z/bias rows
   (w2z, w3z) stay fp16. Weights are pre-scaled by 256 (acts by 16/64) on
   the host to live in the e4m3 normal range; descales fold into the ELU /
   final-copy scalar ops. Layer 1 runs DoubleRow fp8 matmuls (K=256 per
   instruction, 2x PE throughput).

2. Blend-free PSUM accumulation: the per-sample blend coefficient c[b,e]
   is folded into the *stationary* matmul operands (host-scaled hc6/zc3/c6;
   diag-scaled transpose for h1/h2), so the expert sum accumulates directly
   in PSUM. This removes all 24 per-expert DVE blend ops of the old design.
   The transpose+scale of h between layers is a single regular matmul per
   128-chunk against a [32, 6*32] block-of-diagonals operand.
"""

import sys

sys.path.insert(0, "/opt/trn_rl_repo")

import numpy as np
import ml_dtypes

import concourse.bass as bass
import concourse.mybir as mybir
import concourse.tile as tile
from concourse.bass_utils import run_bass_kernel_spmd

B, E = 256, 6
IN, HID, OUT, ZD = 1664, 512, 618, 32
N_CORES = 8
CORE_IDS = list(range(N_CORES))
BC = B // N_CORES         # 32 batch rows per core
K1 = IN // 128            # 13 real k-chunks, layer 1
K1P = 14                  # padded to 14 = 7 DoubleRow pairs (slot 13 zeroed)
KH = HID // 128           # 4 k-chunks for the hidden part of layers 2/3
OUTP = 640                # layer-3 output padded 618 -> 640
NH3 = 2                   # layer-3 output split into halves of 320 (psum bank)
OH3 = OUTP // NH3
ZR = 1 + ZD               # 33 rows: ones row (bias) + z
ZK = 3 * ZR               # 99: three experts' z-blocks stacked per group
FP32 = mybir.dt.float32
FP16 = mybir.dt.float16
FP8 = mybir.dt.float8e4
E4 = ml_dtypes.float8_e4m3
DR = mybir.MatmulPerfMode.DoubleRow

SH0 = 16.0                # h0 pre-scale into fp8
SW = 256.0                # weight pre-scale into fp8
SH = 64.0                 # hidden-act scale (folded into the diag operand)
SZ1 = SH0 * SW            # layer-1 psum scale
SZ2 = SH * SW             # layer-2/3 psum scale


def _split_waits(nc, max_waits=1):
    """neuronxcc walrus accepts only ONE sync-wait per instruction: hoist
    extras onto same-engine NoOps placed before the offending instruction."""
    n = 0
    for fn in nc.m.functions:
        for blk in fn.blocks:
            insts = blk.instructions
            if not any(
                i.sync_info is not None and len(i.sync_info.on_wait) > max_waits
                for i in insts
            ):
                continue
            out = []
            for inst in insts:
                si = inst.sync_info
                if si is not None and len(si.on_wait) > max_waits:
                    for w in si.on_wait[:-max_waits]:
                        n += 1
                        nop = mybir.InstNoOp(name=f"I-wfix{n}", ins=[], outs=[])
                        nop.engine = inst.engine
                        nop.sync_info = mybir.SyncInfo(on_wait=[w], on_update=[])
                        try:
                            nc.register_instruction(nop, overwrite=True)
                        except Exception:
                            pass
                        out.append(nop)
                    inst.sync_info = mybir.SyncInfo(
                        on_wait=list(si.on_wait[-max_waits:]),
                        on_update=list(si.on_update),
                    )
                out.append(inst)
            blk.instructions = out
    return n


def build_nc():
    nc = bass.Bass()

    hc6_d = nc.dram_tensor("hc6", [128, E, K1P, BC], FP8, kind="ExternalInput")
    c6_d = nc.dram_tensor("c6", [E, BC], FP16, kind="ExternalInput")
    zc3_d = nc.dram_tensor("zc3", [ZK, 2, BC], FP16, kind="ExternalInput")
    dg6_d = nc.dram_tensor("dg6", [BC, E * BC], FP16, kind="ExternalInput")
    b1s_d = nc.dram_tensor("b1s", [E, HID], FP16, kind="ExternalInput")
    w1q_d = nc.dram_tensor("w1q", [E, 128, K1, HID], FP8, kind="ExternalInput")
    w2zs_d = nc.dram_tensor("w2zs", [ZK, 2, HID], FP16, kind="ExternalInput")
    w2q_d = nc.dram_tensor("w2q", [E, 128, KH, HID], FP8, kind="ExternalInput")
    w3zs_d = nc.dram_tensor("w3zs", [ZK, 2, OUTP], FP16, kind="ExternalInput")
    w3q_d = nc.dram_tensor("w3q", [E, 128, KH, OUTP], FP8, kind="ExternalInput")
    out_d = nc.dram_tensor("outc", [BC, OUTP], FP32, kind="ExternalOutput")

    with tile.TileContext(nc) as tc:
        with (
            tc.tile_pool(name="const", bufs=1) as cp,
            tc.tile_pool(name="work", bufs=1) as wp,
            tc.tile_pool(name="psum", bufs=2, space="PSUM") as pp,
            tc.tile_pool(name="psumt", bufs=4, space="PSUM") as pt,
        ):
            # ---- DMA schedule. Critical-path-first on the two fast HWDGE
            # queues: layer-1 operands (hc6 on sync; c6/b1s on scalar) ahead
            # of the weight slabs; w1 expert halves interleaved across both
            # queues in PE-consumption order. Slack tensors ride SWDGE.
            hc6 = cp.tile([128, E, K1P, BC], FP8)
            nc.sync.dma_start(out=hc6[:], in_=hc6_d[:])
            c6 = cp.tile([E, BC], FP16)
            nc.scalar.dma_start(out=c6[:], in_=c6_d[:])
            b1s = cp.tile([E, HID], FP16)
            nc.scalar.dma_start(out=b1s[:], in_=b1s_d[:])
            # w1 expert slabs split in k-halves in PE-consumption order:
            # a-halves (k0..7, 0.52MB) on sync, b-halves (k8..12, 0.33MB)
            # on scalar. The scalar queue measurably starts moving ~4us
            # after sync, so the ~3.5/2.0 MB byte split makes both queues
            # finish w1 near-simultaneously (~23us).
            w1 = cp.tile([128, E, K1P, HID], FP8)
            # zero the padded 14th k-slot once; DoubleRow pair 6 reads it
            nc.vector.memset(w1[:, :, K1P - 1, :], 0.0)
            KA = 8
            for e in range(E):
                nc.sync.dma_start(
                    out=w1[:, e, :KA, :], in_=w1q_d[e, :, :KA, :]
                )
                nc.scalar.dma_start(
                    out=w1[:, e, KA:K1, :], in_=w1q_d[e, :, KA:, :]
                )
            # ---- slack tensors (needed only from layer 2 on); the z-path
            # weights go first so the scheduler-hoisted z matmuls never
            # stall the in-order PE stream ----
            zc3 = cp.tile([ZK, 2, BC], FP16)
            nc.gpsimd.dma_start(out=zc3[:], in_=zc3_d[:])
            w2zs = cp.tile([ZK, 2, HID], FP16)
            nc.gpsimd.dma_start(out=w2zs[:], in_=w2zs_d[:])
            w3zs = cp.tile([ZK, 2, OUTP], FP16)
            nc.gpsimd.dma_start(out=w3zs[:], in_=w3zs_d[:])
            dg6 = cp.tile([BC, E * BC], FP16)
            nc.gpsimd.dma_start(out=dg6[:], in_=dg6_d[:])
            w2 = cp.tile([128, E, KH, HID], FP8)
            for e in range(E):
                eng = nc.sync if e % 2 == 0 else nc.scalar
                eng.dma_start(out=w2[:, e, :, :], in_=w2q_d[e])
            w3 = cp.tile([128, E, KH, OUTP], FP8)
            for e in range(E):
                eng = nc.sync if e % 2 == 0 else nc.scalar
                eng.dma_start(out=w3[:, e, :, :], in_=w3q_d[e])

            HH = HID // 2     # 256-column half of a hidden layer

            def elu_half(ps, s, h, c0, tag):
                """h[:, c0:c0+HH] = ELU(ps / s) in fp16 via
                relu(x) + min(exp(x)-1, 0). exp/relu read PSUM directly on
                ACT (descale fused via scale=); x is O(1) so exp is safe."""
                texp = wp.tile([BC, HH], FP16, tag=f"{tag}_exp")
                nc.scalar.activation(
                    texp[:], ps[:], mybir.ActivationFunctionType.Exp,
                    scale=1.0 / s,
                )
                trel = wp.tile([BC, HH], FP16, tag=f"{tag}_rel")
                nc.scalar.activation(
                    trel[:], ps[:], mybir.ActivationFunctionType.Relu,
                    scale=1.0 / s,
                )
                tmin = wp.tile([BC, HH], FP16, tag=f"{tag}_min")
                nc.vector.tensor_scalar(
                    tmin[:], texp[:], -1.0, 0.0,
                    mybir.AluOpType.add, mybir.AluOpType.min,
                )
                nc.vector.tensor_tensor(
                    h[:, c0 : c0 + HH], tmin[:], trel[:], mybir.AluOpType.add
                )

            def tscale_half(h, ht, half, tag):
                """ht[p, e, j, b] = 64 * c[b,e] * h[b, 128j+p] for the two
                128-chunks j of column-half `half`. One regular matmul per
                chunk against the block-of-diags dg6 [32, 6*32] does
                transpose + per-expert coef scaling; PSUM->SBUF fp8 casts
                ride the ACT engine (DVE is on ELU duty)."""
                for j in range(2 * half, 2 * half + 2):
                    ps = pt.tile([128, E, BC], FP32, name=f"{tag}_tp{j}", tag="tpose")
                    nc.tensor.matmul(
                        ps[:], h[:, j * 128 : (j + 1) * 128], dg6[:],
                        start=True, stop=True,
                    )
                    nc.scalar.activation(
                        ht[:, :, j, :], ps[:], mybir.ActivationFunctionType.Copy
                    )

            # All three layers are split into two output-column half-chains
            # (A = first half, B = second half) so the ELU + transpose of
            # half A overlaps PE work of half B — the PE stream never has a
            # multi-us bubble and the gated clock stays at 2.4 GHz.

            # ================= Layer 1 (DoubleRow fp8, 7 pairs) =============
            ps1h = [
                pp.tile([BC, HH], FP32, name=f"l1ps{h}", tag="ps")
                for h in range(2)
            ]
            # Consume w1 in DMA-arrival order: a-halves (pairs 0..3) stream
            # on sync from the start, b-halves (pairs 4..6) on the ~4us-late
            # scalar queue — so a-parts lead by two experts and b-parts
            # trail, keeping PE fed from both queues.
            SA, SB = (0, KA // 2), (KA // 2, K1P // 2)
            L1_ORDER = [
                (0, SA), (1, SA), (0, SB), (2, SA), (1, SB), (3, SA),
                (2, SB), (4, SA), (3, SB), (5, SA), (4, SB), (5, SB),
            ]
            def l1_part(e, j0, j1, h):
                sl = slice(h * HH, (h + 1) * HH)
                for j in range(j0, j1):
                    nc.tensor.matmul(
                        ps1h[h][:],
                        hc6[:, e, 2 * j : 2 * j + 2, :],
                        w1[:, e, 2 * j : 2 * j + 2, sl],
                        start=(e == 0 and j == 0), stop=False,
                        perf_mode=DR,
                    )

            for (e, (j0, j1)) in L1_ORDER[:-2]:
                for h in range(2):
                    l1_part(e, j0, j1, h)
            h1 = wp.tile([BC, HID], FP16, tag="l1_h")
            h1t = wp.tile([128, E, KH, BC], FP8, tag="l1_t")
            # finish chain A fully first: its ELU/transpose latency then
            # hides behind chain B's trailing matmuls.
            for h in range(2):
                sl = slice(h * HH, (h + 1) * HH)
                for (e, (j0, j1)) in L1_ORDER[-2:]:
                    l1_part(e, j0, j1, h)
                # blended bias last: += sum_e c[b,e] * (SZ1*b1[e, half])
                nc.tensor.matmul(
                    ps1h[h][:], c6[:], b1s[:, sl], start=False, stop=True
                )
                elu_half(ps1h[h], SZ1, h1, h * HH, f"l1{h}")
                tscale_half(h1, h1t, h, f"l1{h}")

            # ================= Layer 2 (DoubleRow fp8) =================
            ps2h = [
                pp.tile([BC, HH], FP32, name=f"l2ps{h}", tag="ps")
                for h in range(2)
            ]
            for h in range(2):
                sl = slice(h * HH, (h + 1) * HH)
                for g in range(2):
                    nc.tensor.matmul(
                        ps2h[h][:], zc3[:, g, :], w2zs[:, g, sl],
                        start=(g == 0), stop=False,
                    )
            h2 = wp.tile([BC, HID], FP16, tag="l2_h")
            h2t = wp.tile([128, E, KH, BC], FP8, tag="l2_t")
            # pair j contracts h1 columns [256j, 256j+256) = tscale half j.
            # Expert order [0,2,4,1,3,5]: even slabs ride the early sync
            # queue, odd slabs the late scalar queue. Half A's chain runs
            # fully before half B's tail so ELU-A/T2-A latency hides behind
            # B's matmuls (and ELU-B behind layer 3's pair-0 sweep).
            E_ORDER = [0, 2, 4, 1, 3, 5]
            for h in range(2):
                sl = slice(h * HH, (h + 1) * HH)
                for j in range(KH // 2):
                    for ei, e in enumerate(E_ORDER):
                        nc.tensor.matmul(
                            ps2h[h][:],
                            h1t[:, e, 2 * j : 2 * j + 2, :],
                            w2[:, e, 2 * j : 2 * j + 2, sl],
                            start=False,
                            stop=(ei == E - 1 and j == KH // 2 - 1),
                            perf_mode=DR,
                        )
                elu_half(ps2h[h], SZ2, h2, h * HH, f"l2{h}")
                tscale_half(h2, h2t, h, f"l2{h}")

            # ================= Layer 3 (DoubleRow fp8) =================
            res3 = wp.tile([BC, OUTP], FP32, tag="res3")
            ps3h = [
                pp.tile([BC, OH3], FP32, name=f"l3ps{h}", tag="ps")
                for h in range(2)
            ]
            for h in range(2):
                sl = slice(h * OH3, (h + 1) * OH3)
                for g in range(2):
                    nc.tensor.matmul(
                        ps3h[h][:], zc3[:, g, :], w3zs[:, g, sl],
                        start=(g == 0), stop=False,
                    )
            for h in range(2):
                sl = slice(h * OH3, (h + 1) * OH3)
                for j in range(KH // 2):
                    for ei, e in enumerate(E_ORDER):
                        nc.tensor.matmul(
                            ps3h[h][:],
                            h2t[:, e, 2 * j : 2 * j + 2, :],
                            w3[:, e, 2 * j : 2 * j + 2, sl],
                            start=False,
                            stop=(ei == E - 1 and j == KH // 2 - 1),
                            perf_mode=DR,
                        )
                nc.vector.tensor_scalar(
                    res3[:, sl], ps3h[h][:], 1.0 / SZ2, None,
                    mybir.AluOpType.mult,
                )
                # stream each half out as soon as its copy lands
                eng = nc.scalar if h == 0 else nc.sync
                eng.dma_start(out=out_d[:, sl], in_=res3[:, sl])

    _split_waits(nc)
    _trim_tail(nc)
    return nc


def _trim_tail(nc):
    """Drop the second all-engine barrier round + sem-clear at the kernel
    tail: the first drain+barrier already guarantees completion, and the
    preamble re-initializes semaphores on any re-execution (verified by
    double-execution test)."""
    blk = nc.m.functions[0].blocks[-1]
    insts = blk.instructions
    cut = None
    for idx in range(len(insts) - 1, -1, -1):
        if type(insts[idx]).__name__ == "InstISA":
            cut = idx
            break
    if cut is not None:
        blk.instructions = insts[:cut]


_NC_CACHE = None


def _get_nc():
    global _NC_CACHE
    if _NC_CACHE is None:
        _NC_CACHE = build_nc()
    return _NC_CACHE


def _zgroup(wz, width):
    """[E, 33, width] (bias row + z rows, pre-scaled) -> [99, 2, width]
    where row 33e+r of group g holds expert 3g+e's row r."""
    t = wz.reshape(2, 3, ZR, width).transpose(1, 2, 0, 3)
    return np.ascontiguousarray(t.reshape(ZK, 2, width))


def make_in_maps(p_prev, blending_coef, z, w_l1, b_l1, w_l2, b_l2, w_l3, b_l3):
    f, h = np.float32, np.float16
    h0 = np.concatenate([z, p_prev], axis=1).astype(f)            # [B, IN]
    coef = blending_coef.astype(f)

    w1q = np.ascontiguousarray(                                    # [E,128,K1,HID]
        (SW * w_l1.astype(f)).astype(E4)
        .reshape(E, K1, 128, HID).transpose(0, 2, 1, 3)
    )
    b1s = (SZ1 * b_l1.astype(f)).astype(h)                         # [E, HID]
    w2z = np.concatenate(
        [b_l2.astype(f)[:, None, :], w_l2[:, :ZD, :].astype(f)], axis=1
    )                                                              # [E, 33, HID]
    w2zs = _zgroup((SZ2 * w2z).astype(h), HID)                     # [99, 2, HID]
    w2q = np.ascontiguousarray(                                    # [E,128,KH,HID]
        (SW * w_l2[:, ZD:, :].astype(f)).astype(E4)
        .reshape(E, KH, 128, HID).transpose(0, 2, 1, 3)
    )
    w3p = np.zeros((E, HID + ZD, OUTP), f)
    w3p[:, :, :OUT] = w_l3
    b3p = np.zeros((E, OUTP), f)
    b3p[:, :OUT] = b_l3
    w3z = np.concatenate([b3p[:, None, :], w3p[:, :ZD, :]], axis=1)
    w3zs = _zgroup((SZ2 * w3z).astype(h), OUTP)                    # [99, 2, OUTP]
    w3q = np.ascontiguousarray(                                    # [E,128,KH,OUTP]
        (SW * w3p[:, ZD:, :]).astype(E4)
        .reshape(E, KH, 128, OUTP).transpose(0, 2, 1, 3)
    )
    eye = np.eye(BC, dtype=f)

    in_maps = []
    for c in range(N_CORES):
        bs = slice(c * BC, (c + 1) * BC)
        cc = coef[bs]                                              # [BC, E]
        hv = h0[bs].T.reshape(K1, 128, BC)                         # [k, p, b]
        hc6 = np.zeros((128, E, K1P, BC), E4)                      # k13 stays 0
        hc6[:, :, :K1, :] = (
            SH0 * np.einsum("be,kpb->pekb", cc, hv)
        ).astype(E4)
        c6 = np.ascontiguousarray(cc.T).astype(h)                  # [E, BC]
        zext = np.concatenate(
            [np.ones((BC, 1), f), z[bs].astype(f)], axis=1
        ).T                                                        # [33, BC]
        zc3t = np.einsum("rb,bE->Erb", zext, cc)                   # [6, 33, BC]
        zc3 = np.ascontiguousarray(
            zc3t.reshape(2, 3, ZR, BC).transpose(1, 2, 0, 3).reshape(ZK, 2, BC)
        ).astype(h)
        dg6 = np.ascontiguousarray(                                # [BC, E*BC]
            np.einsum("be,bc->bec", SH * cc, eye).reshape(BC, E * BC)
        ).astype(h)
        in_maps.append(
            {
                "hc6": hc6, "c6": c6, "zc3": zc3, "dg6": dg6, "b1s": b1s,
                "w1q": w1q, "w2zs": w2zs, "w2q": w2q,
                "w3zs": w3zs, "w3q": w3q,
            }
        )
    return in_maps


def assemble_output(results):
    full = np.concatenate(
        [results[c]["outc"] for c in range(N_CORES)], axis=0
    )                                                              # [256, 640]
    return np.ascontiguousarray(full[:, :OUT]).astype(np.float32)


def kernel(p_prev, blending_coef, z, w_l1, b_l1, w_l2, b_l2, w_l3, b_l3):
    args = [
        np.asarray(a)
        for a in (p_prev, blending_coef, z, w_l1, b_l1, w_l2, b_l2, w_l3, b_l3)
    ]
    nc = _get_nc()
    in_maps = make_in_maps(*args)
    res = run_bass_kernel_spmd(nc, in_maps, CORE_IDS)
    return assemble_output(res.results)
